# revision 1
# baseline (speedup 1.0000x reference)
"""GAT (2-layer, PyG-style) on 8 Trainium2 NeuronCores via Bass/Tile.

Strategy (dst-sharded message passing):
  - Destination nodes are partitioned into 8 contiguous chunks (6250/core);
    each core owns all edges incident to its dst chunk, grouped into 128-dst
    windows, each window's edge list split by src half (int16 gather range)
    and padded to 128-edge chunks (uniform across cores -> one SPMD program).
  - Phase 1 (replicated on every core): h_ext = x @ [W1 | W1@Asrc | W1@Adst]
    -> gather table h_tab [npad, 320] holding [h(256) | aS(8) | aD(8) | pad].
  - aD_local [6272, 64]: own-node rows of the aD columns, copied out of
    h_tab with a partition_id-dependent dynamic DMA offset.
  - Phase 2 (per window): gpsimd.dma_gather of h+aS rows by src (two calls:
    low/high table half) and aD rows by local dst; segment softmax +
    weighted aggregation via one-hot S matrices on the PE (denominator as a
    second matmul stream into the same PSUM tile); relu(out+b1) -> h1;
    immediately h2_ext = h1 @ [W2 | W2@a2src | W2@a2dst]; h2 rows -> DRAM,
    aS2/aD2 -> a2_local.
  - AllGather of h2 (the only collective).
  - Phase 3: same windowed machinery for layer 2 (single head); aS2 comes
    from a tensor_tensor_reduce against the gathered h2 rows; log_softmax
    (deferred ln) rows DMA'd to the output.
"""
import sys

for _p in ("/opt/trn_rl_repo", "/opt/pypackages"):
    if _p not in sys.path:
        sys.path.insert(0, _p)

import numpy as np
from concourse import bacc, bass, mybir, tile
from concourse.masks import make_identity

P = 128
F32 = mybir.dt.float32
I16 = mybir.dt.int16
HALF = 32768

# ---- problem constants (nn_GAT_60000693125135) ----
N = 50000
IN_DIM = 256
H1 = 8          # heads layer 1
HID = 32        # per-head dim layer 1
HC1 = H1 * HID  # 256
OUT = 64
NCORES = 8
NEG_SLOPE = 0.2


def _cdiv(a, b):
    return -(-a // b)


def _wrap16(vals, nidx):
    """int16 idx list -> [128, nidx//16] wrap-16 layout, replicated x8."""
    a = np.asarray(vals, np.int16).reshape(nidx // 16, 16).T  # [16, cols]
    return np.tile(a, (8, 1))


# ----------------------------------------------------------------------------
# Host-side preprocessing.
# ----------------------------------------------------------------------------
def prep_edges(edge_index, n, ncores):
    """Shard + window + src-half-split the edge list.

    Returns dict with per-core arrays:
      srclo16 [ncores, 128, 8*CTlo]  (int16, wrap-16 per window)
      srchi16 [ncores, 128, 8*CThi]
      dloc16  [ncores, 128, 8*CT]
      d128    [ncores, 128, CT]     (f32; 999 sentinel on pads)
    and CWlo, CWhi lists (len nw).
    """
    e0 = edge_index[0].astype(np.int64)
    e1 = edge_index[1].astype(np.int64)
    loops = np.arange(n, dtype=np.int64)
    src = np.concatenate([e0, loops])
    dst = np.concatenate([e1, loops])

    nchunk = n // ncores
    nw = _cdiv(nchunk, P)
    core = dst // nchunk
    dloc = dst - core * nchunk
    w = dloc // P
    hi = (src >= HALF).astype(np.int64)
    # group id: (core, window, half)
    gid = (core * nw + w) * 2 + hi
    ngroups = ncores * nw * 2
    cnt = np.bincount(gid, minlength=ngroups).reshape(ncores, nw, 2)
    CWlo = _cdiv(cnt[:, :, 0].max(axis=0), P)          # may be 0
    CWhi = _cdiv(cnt[:, :, 1].max(axis=0), P)
    CW = CWlo + CWhi
    assert CW.min() >= 1
    CTlo, CThi, CT = int(CWlo.sum()), int(CWhi.sum()), int(CW.sum())

    order = np.argsort(gid, kind="stable")
    gid_s = gid[order]
    starts = np.concatenate([[0], np.cumsum(np.bincount(gid_s, minlength=ngroups))])
    pos = np.arange(order.size) - starts[gid_s]

    # slot (within window): lo edges at [0, CWlo*128), hi at [CWlo*128, CW*128)
    c_s = core[order]
    w_s = w[order]
    hi_s = hi[order]
    slot = pos + hi_s * (CWlo[w_s] * P)

    # per-(core,window) slot-value arrays
    srcv = np.zeros((ncores, nw, np.max(CW) * P), np.int64)   # src - half*HALF
    dlocv = np.zeros((ncores, nw, np.max(CW) * P), np.int64)
    d128v = np.full((ncores, nw, np.max(CW) * P), 999.0, np.float32)
    srcv[c_s, w_s, slot] = src[order] - hi_s * HALF
    dlocv[c_s, w_s, slot] = dloc[order]
    d128v[c_s, w_s, slot] = (dloc[order] % P).astype(np.float32)

    srclo16 = np.zeros((ncores, P, 8 * CTlo), np.int16)
    srchi16 = np.zeros((ncores, P, 8 * CThi), np.int16)
    dloc16 = np.zeros((ncores, P, 8 * CT), np.int16)
    d128A = np.full((ncores, P, CT), 999.0, np.float32)
    olo = ohi = oall = 0
    for wi in range(nw):
        nlo, nhi, nall = int(CWlo[wi]) * P, int(CWhi[wi]) * P, int(CW[wi]) * P
        for c in range(ncores):
            if nlo:
                srclo16[c, :, 8 * olo:8 * (olo + nlo // P)] = _wrap16(srcv[c, wi, :nlo], nlo)
            if nhi:
                srchi16[c, :, 8 * ohi:8 * (ohi + nhi // P)] = _wrap16(srcv[c, wi, nlo:nall], nhi)
            dloc16[c, :, 8 * oall:8 * (oall + nall // P)] = _wrap16(dlocv[c, wi, :nall], nall)
            d128A[c, :, oall:oall + nall // P] = d128v[c, wi, :nall].reshape(nall // P, P).T
    # fmt: off
        olo += nlo // P; ohi += nhi // P; oall += nall // P
    # fmt: on
    return dict(srclo16=srclo16, srchi16=srchi16, dloc16=dloc16, d128=d128A,
                CWlo=[int(v) for v in CWlo], CWhi=[int(v) for v in CWhi])


# ----------------------------------------------------------------------------
# Kernel builder (SPMD program, same for all cores).
# ----------------------------------------------------------------------------
def build_nc(cfg):
    n = cfg["N"]; in_dim = cfg["IN"]; hc1 = cfg["HC1"]; h1 = cfg["H1"]
    hid = cfg["HID"]; out_dim = cfg["OUT"]; ncores = cfg["NCORES"]
    neg = cfg["NEG"]
    CWlo, CWhi = cfg["CWlo"], cfg["CWhi"]
    CW = [a + b for a, b in zip(CWlo, CWhi)]

    h2c = 2 * h1
    TROW = hc1 + h2c + (-(hc1 + h2c) % 64)   # gather row f32 count (%64 -> 256B)
    assert out_dim == 64, "h2 gather rows must be 256B"
    nchunk = n // ncores
    nw = _cdiv(nchunk, P)
    assert len(CW) == nw
    CTlo, CThi, CT = sum(CWlo), sum(CWhi), sum(CW)
    ntiles = _cdiv(n, P)
    npad = ntiles * P
    nlpad = _cdiv(nchunk, P) * P             # aD_local rows
    kt1 = _cdiv(in_dim, P)
    ckt = _cdiv(hc1, P)
    cmax = max(CW)
    NB = 8

    nc = bacc.Bacc(None, target_bir_lowering=False, debug=False,
                   num_devices=ncores)

    # ---- I/O ----
    xT_in = nc.dram_tensor("xT", [in_dim, npad], F32, kind="ExternalInput")
    w1_in = nc.dram_tensor("W1", [in_dim, hc1], F32, kind="ExternalInput")
    w1T_in = nc.dram_tensor("W1T", [hc1, in_dim], F32, kind="ExternalInput")
    amat_in = nc.dram_tensor("Amat", [hc1, h2c], F32, kind="ExternalInput")
    w2_in = nc.dram_tensor("W2", [hc1, out_dim], F32, kind="ExternalInput")
    w2T_in = nc.dram_tensor("W2T", [out_dim, hc1], F32, kind="ExternalInput")
    a2_in = nc.dram_tensor("A2", [out_dim, 2], F32, kind="ExternalInput")
    a2s_in = nc.dram_tensor("a2srep", [P, out_dim], F32, kind="ExternalInput")
    b1r_in = nc.dram_tensor("b1r", [P, hc1], F32, kind="ExternalInput")
    b2r_in = nc.dram_tensor("b2r", [P, out_dim], F32, kind="ExternalInput")
    iod_in = nc.dram_tensor("iod", [P, P + CT], F32, kind="ExternalInput")
    slo_in = nc.dram_tensor("srclo16", [P, 8 * CTlo], I16, kind="ExternalInput")
    shi_in = nc.dram_tensor("srchi16", [P, max(8 * CThi, 16)], I16, kind="ExternalInput")
    dlo_in = nc.dram_tensor("dloc16", [P, 8 * CT], I16, kind="ExternalInput")
    out_ext = nc.dram_tensor("out", [nchunk, out_dim], F32, kind="ExternalOutput")

    with tile.TileContext(nc) as tc:
        with (
            tc.tile_pool(name="dram", bufs=1, space="DRAM") as dram,
            tc.tile_pool(name="const", bufs=1) as cpool,
            tc.tile_pool(name="xst", bufs=2) as xpool,
            tc.tile_pool(name="hst", bufs=2) as hpool,
            tc.tile_pool(name="gbuf", bufs=2) as gpool,
            tc.tile_pool(name="sbuf2", bufs=2) as spool,
            tc.tile_pool(name="small", bufs=3) as smpool,
            tc.tile_pool(name="adl", bufs=8) as adpool,
            tc.tile_pool(name="w0", bufs=1) as w0pool,
            tc.tile_pool(name="psA", bufs=2, space="PSUM") as psA,
            tc.tile_pool(name="psB", bufs=2, space="PSUM") as psB,
            tc.tile_pool(name="psC", bufs=2, space="PSUM") as psC,
        ):
            # ---- DRAM scratch ----
            h_tab = dram.tile([npad, TROW], F32)
            aD_local = dram.tile([nlpad, 64], F32)
            a2_local = dram.tile([nlpad, 64], F32)
            h2_mine = dram.tile([nchunk, out_dim], F32)
            cc_space = "Shared" if ncores > 4 else "Local"
            h2_tab = dram.tile([n, out_dim], F32, addr_space=cc_space)

            # ---- resident constants ----
            iod_t = cpool.tile([P, P + CT], F32)
            nc.sync.dma_start(out=iod_t[:], in_=iod_in[:])
            iota_t = iod_t[:, 0:P]
            d128t = iod_t[:, P:]
            ident = cpool.tile([P, P], F32)
            make_identity(nc, ident[:])
            b1r = cpool.tile([P, hc1], F32)
            nc.sync.dma_start(out=b1r[:], in_=b1r_in[:])
            b2r = cpool.tile([P, out_dim], F32)
            nc.sync.dma_start(out=b2r[:], in_=b2r_in[:])
            a2srep = cpool.tile([P, out_dim], F32)
            nc.sync.dma_start(out=a2srep[:], in_=a2s_in[:])
            slo = cpool.tile([P, 8 * CTlo], I16)
            nc.sync.dma_start(out=slo[:], in_=slo_in[:])
            shi = cpool.tile([P, max(8 * CThi, 16)], I16)
            nc.sync.dma_start(out=shi[:], in_=shi_in[:])
            dlo = cpool.tile([P, 8 * CT], I16)
            nc.sync.dma_start(out=dlo[:], in_=dlo_in[:])

            # ---- phase 0: extended weights ----
            w1ext = cpool.tile([P, kt1, hc1 + h2c], F32)
            for kt in range(kt1):
                kp = min(P, in_dim - kt * P)
                nc.sync.dma_start(out=w1ext[:kp, kt, 0:hc1], in_=w1_in[kt * P:kt * P + kp, :])
            w1T_sb = w0pool.tile([P, ckt, in_dim], F32)
            amat_sb = w0pool.tile([P, ckt, h2c], F32)
            for c in range(ckt):
                cp = min(P, hc1 - c * P)
                nc.sync.dma_start(out=w1T_sb[:cp, c, :], in_=w1T_in[c * P:c * P + cp, :])
                nc.sync.dma_start(out=amat_sb[:cp, c, :], in_=amat_in[c * P:c * P + cp, :])
            for kt in range(kt1):
                kp = min(P, in_dim - kt * P)
                wps = psB.tile([P, h2c], F32, tag="tp")
                for c in range(ckt):
                    cp = min(P, hc1 - c * P)
                    nc.tensor.matmul(out=wps[:kp, :], lhsT=w1T_sb[:cp, c, kt * P:kt * P + kp],
                                     rhs=amat_sb[:cp, c, :], start=(c == 0), stop=(c == ckt - 1))
                nc.scalar.copy(out=w1ext[:kp, kt, hc1:], in_=wps[:kp, :])

            w2ext = cpool.tile([P, ckt, out_dim + 2], F32)
            w2T_sb = w0pool.tile([out_dim, hc1], F32)
            a2_sb = w0pool.tile([out_dim, 2], F32)
            nc.sync.dma_start(out=w2T_sb[:], in_=w2T_in[:])
            nc.sync.dma_start(out=a2_sb[:], in_=a2_in[:])
            for c in range(ckt):
                cp = min(P, hc1 - c * P)
                nc.sync.dma_start(out=w2ext[:cp, c, 0:out_dim], in_=w2_in[c * P:c * P + cp, :])
                wps2 = psB.tile([P, 2], F32, tag="tp")
                nc.tensor.matmul(out=wps2[:cp, :], lhsT=w2T_sb[:, c * P:c * P + cp],
                                 rhs=a2_sb[:], start=True, stop=True)
                nc.scalar.copy(out=w2ext[:cp, c, out_dim:], in_=wps2[:cp, :])

            # ---- phase 1: h_ext = x @ w1ext -> h_tab [npad, TROW] ----
            for g in range(_cdiv(ntiles, NB)):
                nt0 = g * NB
                nb = min(NB, ntiles - nt0)
                xst = xpool.tile([P, kt1, NB * P], F32, tag="xst")
                for kt in range(kt1):
                    kp = min(P, in_dim - kt * P)
                    nc.sync.dma_start(out=xst[:kp, kt, 0:nb * P],
                                      in_=xT_in[kt * P:kt * P + kp, nt0 * P:nt0 * P + nb * P])
                hstg = hpool.tile([P, NB, hc1 + h2c], F32, tag="hst")
                for j in range(nb):
                    ps = psA.tile([P, hc1 + h2c], F32, tag="mm")
                    for kt in range(kt1):
                        kp = min(P, in_dim - kt * P)
                        nc.tensor.matmul(out=ps[:], lhsT=xst[:kp, kt, j * P:(j + 1) * P],
                                         rhs=w1ext[:kp, kt, :], start=(kt == 0), stop=(kt == kt1 - 1))
                    nc.scalar.copy(out=hstg[:, j, :], in_=ps[:])
                hv = h_tab[nt0 * P:(nt0 + nb) * P, 0:hc1 + h2c].rearrange(
                    "(j p) c -> p j c", p=P)
                nc.sync.dma_start(out=hv, in_=hstg[:, 0:nb, :])

            # ---- aD_local: own-node aD rows via dynamic-offset DMA ----
            pid_rows = nc.sync.snap(nc.sync.partition_id() * nchunk)
            for w in range(nw):
                rows = min(P, nchunk - w * P)
                bnc = adpool.tile([P, h1], F32, tag="adl")
                nc.sync.dma_start(
                    out=bnc[:rows, :],
                    in_=h_tab[bass.ds(pid_rows + w * P, rows), hc1 + h1:hc1 + h2c])
                nc.sync.dma_start(out=aD_local[w * P:w * P + rows, 0:h1], in_=bnc[:rows, :])

            stop = cfg.get("STOP", "")

            def bounce_out(src_dram):
                for w in range(nw):
                    rows = min(P, nchunk - w * P)
                    dbg = smpool.tile([P, out_dim], F32, tag="z")
                    nc.sync.dma_start(out=dbg[:rows, :],
                                      in_=src_dram[w * P:w * P + rows, 0:out_dim])
                    nc.sync.dma_start(out=out_ext[w * P:w * P + rows, :],
                                      in_=dbg[:rows, :])

            if stop == "phase1":
                bounce_out(h_tab)
                return nc

            # ---- phase 2: layer-1 edge aggregation per dst window ----
            olo = oall = 0
            for w in range(nw):
                Clo, Chi, C = CWlo[w], CWhi[w], CW[w]
                rows = min(P, nchunk - w * P)
                G = gpool.tile([P, cmax, TROW], F32, tag="G")
                if Clo:
                    nc.gpsimd.dma_gather(
                        out_ap=G[:, 0:Clo, :], in_ap=h_tab[:],
                        idxs_ap=slo[:, 8 * olo:8 * (olo + Clo)],
                        num_idxs=Clo * P, num_idxs_reg=Clo * P, elem_size=TROW,
                        single_packet=False)
                if Chi:
                    nc.gpsimd.dma_gather(
                        out_ap=G[:, Clo:C, :], in_ap=h_tab[HALF:, :],
                        idxs_ap=shi[:, 8 * (oall - olo):8 * (oall - olo + Chi)],
                        num_idxs=Chi * P, num_idxs_reg=Chi * P, elem_size=TROW,
                        single_packet=False)
                aDb = spool.tile([P, cmax, 64], F32, tag="aDb")
                nc.gpsimd.dma_gather(
                    out_ap=aDb[:, 0:C, :], in_ap=aD_local[:],
                    idxs_ap=dlo[:, 8 * oall:8 * (oall + C)],
                    num_idxs=C * P, num_idxs_reg=C * P, elem_size=64,
                    single_packet=False)
                if stop == "gather":
                    nc.sync.dma_start(out=out_ext[w * P:w * P + rows, :],
                                      in_=G[:rows, 0, 0:out_dim])
                    olo += Clo; oall += C
                    continue
                S = spool.tile([P, cmax, P], F32, tag="S")
                nc.vector.tensor_tensor(
                    out=S[:, 0:C, :],
                    in0=d128t[:, oall:oall + C].unsqueeze(-1).to_broadcast((P, C, P)),
                    in1=iota_t.unsqueeze(1).to_broadcast((P, C, P)),
                    op=mybir.AluOpType.is_equal)
                # p = exp(lrelu(aS + aD)), written back over the aS columns
                # of G so one matmul covers features + denominator.
                nc.vector.tensor_add(out=G[:, 0:C, hc1:hc1 + h1],
                                     in0=G[:, 0:C, hc1:hc1 + h1],
                                     in1=aDb[:, 0:C, 0:h1])
                nc.vector.scalar_tensor_tensor(
                    out=G[:, 0:C, hc1:hc1 + h1], in0=G[:, 0:C, hc1:hc1 + h1],
                    scalar=neg, in1=G[:, 0:C, hc1:hc1 + h1],
                    op0=mybir.AluOpType.mult, op1=mybir.AluOpType.max)
                nc.scalar.activation(out=G[:, 0:C, hc1:hc1 + h1],
                                     in_=G[:, 0:C, hc1:hc1 + h1],
                                     func=mybir.ActivationFunctionType.Exp)
                for h in range(h1):
                    nc.vector.tensor_tensor(
                        out=G[:, 0:C, h * hid:(h + 1) * hid],
                        in0=G[:, 0:C, h * hid:(h + 1) * hid],
                        in1=G[:, 0:C, hc1 + h:hc1 + h + 1].to_broadcast((P, C, hid)),
                        op=mybir.AluOpType.mult)
                ops = psA.tile([P, hc1 + h1], F32, tag="mm")
                for k in range(C):
                    nc.tensor.matmul(out=ops[:], lhsT=S[:, k, :], rhs=G[:, k, 0:hc1 + h1],
                                     start=(k == 0), stop=(k == C - 1))
                rec = smpool.tile([P, h1], F32, tag="rec")
                nc.vector.reciprocal(out=rec[:], in_=ops[:, hc1:hc1 + h1])
                h1w = spool.tile([P, hc1], F32, tag="h1w")
                nc.vector.tensor_tensor(
                    out=h1w[:].rearrange("p (h j) -> p h j", h=h1),
                    in0=ops[:, 0:hc1].rearrange("p (h j) -> p h j", h=h1),
                    in1=rec[:].unsqueeze(-1).to_broadcast((P, h1, hid)),
                    op=mybir.AluOpType.mult)
                nc.vector.tensor_add(out=h1w[:], in0=h1w[:], in1=b1r[:])
                nc.vector.tensor_scalar(out=h1w[:], in0=h1w[:], scalar1=0.0, scalar2=None,
                                        op0=mybir.AluOpType.max)
                if stop == "smm":
                    nc.sync.dma_start(out=out_ext[w * P:w * P + rows, :],
                                      in_=h1w[:rows, 0:out_dim])
                    olo += Clo; oall += C
                    continue
                # layer-2 row prep: h2_ext = h1 @ w2ext
                h1T = spool.tile([P, ckt, P], F32, tag="h1T")
                for c in range(ckt):
                    cp = min(P, hc1 - c * P)
                    tp = psB.tile([P, P], F32, tag="tp")
                    nc.tensor.transpose(tp[:cp, :], h1w[:, c * P:c * P + cp], ident[:])
                    nc.scalar.copy(out=h1T[:cp, c, :], in_=tp[:cp, :])
                h2ps = psC.tile([P, out_dim + 2], F32, tag="h2")
                for c in range(ckt):
                    cp = min(P, hc1 - c * P)
                    nc.tensor.matmul(out=h2ps[:], lhsT=h1T[:cp, c, :], rhs=w2ext[:cp, c, :],
                                     start=(c == 0), stop=(c == ckt - 1))
                h2sb = smpool.tile([P, out_dim + 2], F32, tag="h2sb")
                nc.scalar.copy(out=h2sb[:], in_=h2ps[:])
                nc.sync.dma_start(out=h2_mine[w * P:w * P + rows, :], in_=h2sb[:rows, 0:out_dim])
                nc.sync.dma_start(out=a2_local[w * P:w * P + rows, 0:2],
                                  in_=h2sb[:rows, out_dim:])
                olo += Clo; oall += C

            if stop in ("gather", "smm"):
                return nc
            if stop == "phase2":
                bounce_out(h2_mine)
                return nc

            # ---- all-gather h2 ----
            nc.gpsimd.collective_compute(
                "AllGather", mybir.AluOpType.bypass,
                replica_groups=[list(range(ncores))],
                ins=[h2_mine[:].opt()], outs=[h2_tab[:].opt()])

            if stop == "cc":
                bounce_out(h2_tab)
                return nc

            # ---- phase 3: layer-2 edge aggregation + log_softmax ----
            t_all = cpool.tile([P, nw, out_dim], F32)
            s_all = cpool.tile([P, nw], F32)
            olo = oall = 0
            for w in range(nw):
                Clo, Chi, C = CWlo[w], CWhi[w], CW[w]
                rows = min(P, nchunk - w * P)
                G2 = gpool.tile([P, cmax, out_dim], F32, tag="G")
                if Clo:
                    nc.gpsimd.dma_gather(
                        out_ap=G2[:, 0:Clo, :], in_ap=h2_tab[:],
                        idxs_ap=slo[:, 8 * olo:8 * (olo + Clo)],
                        num_idxs=Clo * P, num_idxs_reg=Clo * P, elem_size=out_dim,
                        single_packet=False)
                if Chi:
                    nc.gpsimd.dma_gather(
                        out_ap=G2[:, Clo:C, :], in_ap=h2_tab[HALF:, :],
                        idxs_ap=shi[:, 8 * (oall - olo):8 * (oall - olo + Chi)],
                        num_idxs=Chi * P, num_idxs_reg=Chi * P, elem_size=out_dim,
                        single_packet=False)
                aDb2 = spool.tile([P, cmax, 64], F32, tag="aDb")
                nc.gpsimd.dma_gather(
                    out_ap=aDb2[:, 0:C, :], in_ap=a2_local[:],
                    idxs_ap=dlo[:, 8 * oall:8 * (oall + C)],
                    num_idxs=C * P, num_idxs_reg=C * P, elem_size=64,
                    single_packet=False)
                S = spool.tile([P, cmax, P], F32, tag="S")
                nc.vector.tensor_tensor(
                    out=S[:, 0:C, :],
                    in0=d128t[:, oall:oall + C].unsqueeze(-1).to_broadcast((P, C, P)),
                    in1=iota_t.unsqueeze(1).to_broadcast((P, C, P)),
                    op=mybir.AluOpType.is_equal)
                # aS2_e = sum_c G2[e,:,c]*a2src[c]
                tmp2 = spool.tile([P, cmax, out_dim], F32, tag="tmp2")
                nc.vector.tensor_tensor(
                    out=tmp2[:, 0:C, :], in0=G2[:, 0:C, :],
                    in1=a2srep[:].unsqueeze(1).to_broadcast((P, C, out_dim)),
                    op=mybir.AluOpType.mult)
                p2 = smpool.tile([P, cmax], F32, tag="pe")
                nc.vector.tensor_reduce(out=p2[:, 0:C], in_=tmp2[:, 0:C, :],
                                        axis=mybir.AxisListType.X,
                                        op=mybir.AluOpType.add)
                nc.vector.tensor_add(out=p2[:, 0:C], in0=p2[:, 0:C],
                                     in1=aDb2[:, 0:C, 1].squeeze())
                nc.vector.scalar_tensor_tensor(
                    out=p2[:, 0:C], in0=p2[:, 0:C], scalar=neg, in1=p2[:, 0:C],
                    op0=mybir.AluOpType.mult, op1=mybir.AluOpType.max)
                nc.scalar.activation(out=p2[:, 0:C], in_=p2[:, 0:C],
                                     func=mybir.ActivationFunctionType.Exp)
                nc.vector.tensor_tensor(
                    out=G2[:, 0:C, :], in0=G2[:, 0:C, :],
                    in1=p2[:, 0:C].unsqueeze(-1).to_broadcast((P, C, out_dim)),
                    op=mybir.AluOpType.mult)
                ops2 = psA.tile([P, out_dim], F32, tag="mm")
                den2 = psA.tile([P, 1], F32, tag="den")
                for k in range(C):
                    nc.tensor.matmul(out=ops2[:], lhsT=S[:, k, :], rhs=G2[:, k, :],
                                     start=(k == 0), stop=(k == C - 1))
                    nc.tensor.matmul(out=den2[:], lhsT=S[:, k, :],
                                     rhs=p2[:, k:k + 1],
                                     start=(k == 0), stop=(k == C - 1))
                rec2 = smpool.tile([P, 1], F32, tag="rec")
                nc.vector.reciprocal(out=rec2[:], in_=den2[:])
                z = smpool.tile([P, out_dim], F32, tag="z")
                nc.vector.tensor_tensor(out=z[:], in0=ops2[:],
                                        in1=rec2[:].to_broadcast((P, out_dim)),
                                        op=mybir.AluOpType.mult)
                nc.vector.tensor_add(out=z[:], in0=z[:], in1=b2r[:])
                negmax = smpool.tile([P, 1], F32, tag="rec")
                nc.vector.tensor_reduce(out=negmax[:], in_=z[:], axis=mybir.AxisListType.X,
                                        op=mybir.AluOpType.max, negate=True)
                nc.vector.tensor_scalar(out=t_all[:, w, :], in0=z[:], scalar1=negmax[:],
                                        scalar2=None, op0=mybir.AluOpType.add)
                esc = smpool.tile([P, out_dim], F32, tag="z")
                nc.scalar.activation(out=esc[:], in_=t_all[:, w, :],
                                     func=mybir.ActivationFunctionType.Exp,
                                     accum_out=s_all[:, w:w + 1])
                olo += Clo; oall += C
            # epilogue: res = t - ln(s)
            lns = cpool.tile([P, nw], F32)
            nc.scalar.activation(out=lns[:], in_=s_all[:],
                                 func=mybir.ActivationFunctionType.Ln)
            for w in range(nw):
                rows = min(P, nchunk - w * P)
                res = smpool.tile([P, out_dim], F32, tag="z")
                nc.vector.tensor_scalar(out=res[:], in0=t_all[:, w, :], scalar1=lns[:, w:w + 1],
                                        scalar2=None, op0=mybir.AluOpType.subtract)
                nc.sync.dma_start(out=out_ext[w * P:w * P + rows, :], in_=res[:rows, :])

    return nc


# ----------------------------------------------------------------------------
# Host-side input packing.
# ----------------------------------------------------------------------------
def make_in_maps(inputs, cfg):
    n = cfg["N"]; in_dim = cfg["IN"]; hc1 = cfg["HC1"]; h1 = cfg["H1"]
    hid = cfg["HID"]; ncores = cfg["NCORES"]

    x = np.asarray(inputs["x"], np.float32)
    ei = np.asarray(inputs["edge_index"])
    W1 = np.asarray(inputs["W1"], np.float32)
    a_src1 = np.asarray(inputs["a_src1"], np.float32)
    a_dst1 = np.asarray(inputs["a_dst1"], np.float32)
    b1 = np.asarray(inputs["b1"], np.float32)
    W2 = np.asarray(inputs["W2"], np.float32)
    a_src2 = np.asarray(inputs["a_src2"], np.float32)
    a_dst2 = np.asarray(inputs["a_dst2"], np.float32)
    b2 = np.asarray(inputs["b2"], np.float32)

    ntiles = _cdiv(n, P)
    npad = ntiles * P
    xT = np.zeros((in_dim, npad), np.float32)
    xT[:, :n] = x.T

    amat = np.zeros((hc1, 2 * h1), np.float32)
    for h in range(h1):
        amat[h * hid:(h + 1) * hid, h] = a_src1[h]
        amat[h * hid:(h + 1) * hid, h1 + h] = a_dst1[h]

    a2 = np.stack([a_src2[0], a_dst2[0]], axis=1).astype(np.float32)

    pe = prep_edges(ei, n, ncores)
    cfg["CWlo"], cfg["CWhi"] = pe["CWlo"], pe["CWhi"]
    CT = sum(cfg["CWlo"]) + sum(cfg["CWhi"])

    common = {
        "W1": W1, "W1T": np.ascontiguousarray(W1.T),
        "Amat": amat, "W2": W2, "W2T": np.ascontiguousarray(W2.T), "A2": a2,
        "a2srep": np.tile(a_src2[0][None, :], (P, 1)).astype(np.float32),
        "b1r": np.tile(b1[None, :], (P, 1)).astype(np.float32),
        "b2r": np.tile(b2[None, :], (P, 1)).astype(np.float32),
        "xT": xT,
    }
    iota = np.tile(np.arange(P, dtype=np.float32)[None, :], (P, 1))
    in_maps = []
    for c in range(ncores):
        m = dict(common)
        m["srclo16"] = np.ascontiguousarray(pe["srclo16"][c])
        shi = pe["srchi16"][c]
        if shi.shape[1] == 0:
            shi = np.zeros((P, 16), np.int16)
        m["srchi16"] = np.ascontiguousarray(shi)
        m["dloc16"] = np.ascontiguousarray(pe["dloc16"][c])
        m["iod"] = np.ascontiguousarray(
            np.concatenate([iota, pe["d128"][c]], axis=1))
        in_maps.append(m)
    return in_maps


DEFAULT_CFG = dict(N=N, IN=IN_DIM, HC1=HC1, H1=H1, HID=HID, OUT=OUT,
                   NCORES=NCORES, NEG=NEG_SLOPE)

TRACE = False
LAST_RESULTS = None


def kernel(**inputs) -> np.ndarray:
    global LAST_RESULTS
    from concourse.bass_utils import run_bass_kernel_spmd

    cfg = dict(DEFAULT_CFG)
    in_maps = make_in_maps(inputs, cfg)
    nc = build_nc(cfg)
    if not nc.is_finalized():
        nc.finalize()
    res = run_bass_kernel_spmd(nc, in_maps, core_ids=list(range(cfg["NCORES"])),
                               trace=TRACE)
    LAST_RESULTS = res
    outs = [res.results[c]["out"] for c in range(cfg["NCORES"])]
    return np.concatenate(outs, axis=0)



# revision 7
# speedup vs baseline: 1.1192x; 1.1192x over previous
"""GAT (2-layer, PyG-style) on 8 Trainium2 NeuronCores via Bass/Tile.

v2: dst-aligned slot layout + fp16 datapath.

  - Nodes are globally sorted by (lo_indeg//3, hi_indeg) and dealt
    round-robin to (window, row, core): window w, partition-row p of core c
    holds node order[w*1024 + p*8 + c].  All 8 cores see statistically
    identical windows, so the SPMD-uniform per-window slot counts
    (Clo[w]/Chi[w] = max per-half indegree over rows and cores) stay tight
    (~+18% padding).  Dealing also makes each core's row index q equal the
    node-order row of its chunk, so the h2 table needs no unpermute pass.
  - Edge slots are dst-row aligned: window w, chunk k, partition p holds
    the k-th in-edge of the node at row p (lo chunks [0,Clo), hi chunks
    [Clo,C)).  The per-edge gather brings the src row of h_tab (256 fp16
    = 512B).  Everything downstream is per-partition independent:
    attention logits via DVE segmented reduce, aD via a per-window
    [128,8] tile (no per-edge dst gather), segment softmax via free-dim
    reduce, aggregation via a chunk-accumulate DVE loop (no one-hot
    matmuls).
  - Layer 2 repeats the scheme with its own node ordering (the lo/hi
    split by h2-table row differs from layer 1), gathering 256B rows
    [h2(64) | aS2 | aD2 | pad] so the src attention term rides along.
  - One AllGather of h2 (node-ordered 256B rows); log_softmax in f32.
"""
import sys

for _p in ("/opt/trn_rl_repo", "/opt/pypackages"):
    if _p not in sys.path:
        sys.path.insert(0, _p)

import numpy as np
from concourse import bacc, bass, mybir, tile
from concourse.masks import make_identity

P = 128
F32 = mybir.dt.float32
F16 = mybir.dt.float16
I16 = mybir.dt.int16
HALF = 32768

# ---- problem constants (nn_GAT_60000693125135) ----
N = 50000
IN_DIM = 256
H1 = 8          # heads layer 1
HID = 32        # per-head dim layer 1
HC1 = H1 * HID  # 256
OUT = 64
NCORES = 8
NEG_SLOPE = 0.2
NCHUNK = N // NCORES            # 6250
NW = -(-NCHUNK // P)            # 49
NLPAD = NW * P                  # 6272
NPAD = -(-N // P) * P           # 50176
ZHI = N + 100 - HALF            # a zero row in aDfull's hi half


def _cdiv(a, b):
    return -(-a // b)


def _wrap16(vals):
    """int16 idx stream (len % 16 == 0) -> [128, len//16] wrap-16 layout."""
    a = np.asarray(vals, np.int16).reshape(-1, 16).T
    return np.tile(a, (8, 1))


# ----------------------------------------------------------------------------
# Host-side preprocessing.
# ----------------------------------------------------------------------------
def _plan(lo_cnt, hi_cnt, band=3):
    """Global sort by (lo//band, hi); deal to (w, p, c).

    Returns g_of_node (node -> global sorted pos), Clo[NW], Chi[NW]."""
    order = np.lexsort((hi_cnt, lo_cnt // band))
    g_of = np.empty(N, np.int64)
    g_of[order] = np.arange(N)
    lo_s = np.zeros(NW * P * NCORES, np.int64)
    hi_s = np.zeros(NW * P * NCORES, np.int64)
    lo_s[:N] = lo_cnt[order]
    hi_s[:N] = hi_cnt[order]
    Clo = lo_s.reshape(NW, P * NCORES).max(1)
    Chi = hi_s.reshape(NW, P * NCORES).max(1)
    return g_of, [int(v) for v in Clo], [int(v) for v in Chi]


def _fill_slots(src_val, dst_node, half, g_of, Clo, Chi):
    """Per-core wrapped idx streams + mask for one layer.

    Slot for the k-th (by edge order) half-edge of the node at (w, p, c):
    lo stream pos (olo[w]+k)*128+p, hi stream pos (ohi[w]+k)*128+p.
    Returns slo[nc], shi[nc], mask[nc][128, CT] (f16)."""
    CTlo, CThi = sum(Clo), sum(Chi)
    CW = [a + b for a, b in zip(Clo, Chi)]
    CT = sum(CW)
    olo = np.concatenate([[0], np.cumsum(Clo)])[:-1]
    ohi = np.concatenate([[0], np.cumsum(Chi)])[:-1]
    oall = np.concatenate([[0], np.cumsum(CW)])[:-1]

    g = g_of[dst_node]
    c = g % NCORES
    p = (g // NCORES) % P
    w = g // (P * NCORES)

    # occurrence index within (dst node, half)
    key = g * 2 + half
    sort = np.argsort(key, kind="stable")
    ks = key[sort]
    starts = np.concatenate([[0], np.cumsum(np.bincount(ks, minlength=2 * N))])
    occ = np.arange(key.size) - starts[ks]
    k_of = np.empty_like(occ)
    k_of[sort] = occ

    lo_pos = (olo[w] + k_of) * P + p
    hi_pos = (ohi[w] + k_of) * P + p
    m_pos = (oall[w] + np.where(half == 0, k_of, np.asarray(Clo)[w] + k_of)) * P + p

    slo, shi, mask = [], [], []
    for ci in range(NCORES):
        mc = c == ci
        ml = mc & (half == 0)
        mh = mc & (half == 1)
        lo_flat = np.zeros(max(CTlo, 1) * P, np.int16)
        hi_flat = np.zeros(max(CThi, 1) * P, np.int16)
        m_flat = np.zeros(CT * P, np.float16)
        lo_flat[lo_pos[ml]] = src_val[ml].astype(np.int16)
        hi_flat[hi_pos[mh]] = src_val[mh].astype(np.int16)
        m_flat[m_pos[mc]] = 1.0
        slo.append(_wrap16(lo_flat))
        shi.append(_wrap16(hi_flat))
        mask.append(np.ascontiguousarray(m_flat.reshape(CT, P).T))
    return slo, shi, mask


def prep_edges(edge_index):
    e0 = edge_index[0].astype(np.int64)
    e1 = edge_index[1].astype(np.int64)
    loops = np.arange(N, dtype=np.int64)
    src = np.concatenate([e0, loops])
    dst = np.concatenate([e1, loops])

    # ---- layer 1: split by src < HALF ----
    half1 = (src >= HALF).astype(np.int64)
    lo1 = np.bincount(dst[half1 == 0], minlength=N)
    hi1 = np.bincount(dst[half1 == 1], minlength=N)
    gA, Clo1, Chi1 = _plan(lo1, hi1)
    slo1, shi1, mask1 = _fill_slots(src - half1 * HALF, dst, half1, gA, Clo1, Chi1)

    cA = gA % NCORES
    qA = (gA // NCORES) % P + (gA // (P * NCORES)) * P   # < NCHUNK
    h2row = cA * NCHUNK + qA                             # node's h2_tab row

    # ---- layer 2: split by h2row < HALF ----
    half2 = (h2row[src] >= HALF).astype(np.int64)
    lo2 = np.bincount(dst[half2 == 0], minlength=N)
    hi2 = np.bincount(dst[half2 == 1], minlength=N)
    gB, Clo2, Chi2 = _plan(lo2, hi2)
    slo2, shi2, mask2 = _fill_slots(h2row[src] - half2 * HALF, dst, half2,
                                    gB, Clo2, Chi2)
    cB = gB % NCORES
    qB = (gB // NCORES) % P + (gB // (P * NCORES)) * P

    # ---- aux idx streams (per core, len NLPAD, trailing rows dummy) ----
    adlo = np.zeros((NCORES, NLPAD), np.int16)
    adhi = np.full((NCORES, NLPAD), ZHI, np.int16)
    mlo = np.zeros((NCORES, NLPAD), np.float16)
    a2lo = np.zeros((NCORES, NLPAD), np.int16)
    a2hi = np.zeros((NCORES, NLPAD), np.int16)
    m2lo = np.zeros((NCORES, NLPAD), np.float16)
    m2hi = np.zeros((NCORES, NLPAD), np.float16)
    nodes = np.arange(N)
    # layer-1 aD extraction: A-row q of core c holds node (cA==c, qA==q)
    is_lo = nodes < HALF
    adlo[cA[is_lo], qA[is_lo]] = nodes[is_lo].astype(np.int16)
    mlo[cA[is_lo], qA[is_lo]] = 1.0
    adhi[cA[~is_lo], qA[~is_lo]] = (nodes[~is_lo] - HALF).astype(np.int16)
    # layer-2 a2 extraction from h2_tab: B-row q of core c holds node with
    # (cB==c, qB==q); its h2_tab row is h2row[node].
    is_lo2 = h2row < HALF
    a2lo[cB[is_lo2], qB[is_lo2]] = h2row[is_lo2].astype(np.int16)
    m2lo[cB[is_lo2], qB[is_lo2]] = 1.0
    a2hi[cB[~is_lo2], qB[~is_lo2]] = (h2row[~is_lo2] - HALF).astype(np.int16)
    m2hi[cB[~is_lo2], qB[~is_lo2]] = 1.0

    return dict(
        Clo1=Clo1, Chi1=Chi1, Clo2=Clo2, Chi2=Chi2,
        slo1=slo1, shi1=shi1, mask1=mask1,
        slo2=slo2, shi2=shi2, mask2=mask2,
        adlo=adlo, adhi=adhi, mlo=mlo,
        a2lo=a2lo, a2hi=a2hi, m2lo=m2lo, m2hi=m2hi,
        cB=cB, qB=qB,
    )


# ----------------------------------------------------------------------------
# Kernel builder (SPMD program, same for all cores).
# ----------------------------------------------------------------------------
def build_nc(cfg):
    neg = NEG_SLOPE
    Clo1, Chi1 = cfg["Clo1"], cfg["Chi1"]
    Clo2, Chi2 = cfg["Clo2"], cfg["Chi2"]
    CW1 = [a + b for a, b in zip(Clo1, Chi1)]
    CW2 = [a + b for a, b in zip(Clo2, Chi2)]
    CT1, CT2 = sum(CW1), sum(CW2)
    CTlo1, CThi1 = sum(Clo1), sum(Chi1)
    CTlo2, CThi2 = sum(Clo2), sum(Chi2)
    cmax1, cmax2 = max(CW1), max(CW2)
    ntiles = NPAD // P
    NB = 8
    kt1 = IN_DIM // P     # 2
    ckt = HC1 // P        # 2
    stop = cfg.get("STOP", "")

    nc = bacc.Bacc(None, target_bir_lowering=False, debug=False,
                   num_devices=NCORES)

    # ---- I/O ----
    xT_in = nc.dram_tensor("xT", [IN_DIM, NPAD], F16, kind="ExternalInput")
    w1_in = nc.dram_tensor("W1", [IN_DIM, HC1], F16, kind="ExternalInput")
    w1T_in = nc.dram_tensor("W1T", [HC1, IN_DIM], F16, kind="ExternalInput")
    amat_in = nc.dram_tensor("Amat", [HC1, H1], F16, kind="ExternalInput")
    w2_in = nc.dram_tensor("W2", [HC1, OUT], F16, kind="ExternalInput")
    w2T_in = nc.dram_tensor("W2T", [OUT, HC1], F16, kind="ExternalInput")
    a2_in = nc.dram_tensor("A2", [OUT, 2], F16, kind="ExternalInput")
    asr_in = nc.dram_tensor("asr", [P, HC1], F16, kind="ExternalInput")
    b1r_in = nc.dram_tensor("b1r", [P, HC1], F16, kind="ExternalInput")
    b2r_in = nc.dram_tensor("b2r", [P, OUT], F32, kind="ExternalInput")
    m1_in = nc.dram_tensor("mask1", [P, CT1], F16, kind="ExternalInput")
    m2_in = nc.dram_tensor("mask2", [P, CT2], F16, kind="ExternalInput")
    slo1_in = nc.dram_tensor("slo1", [P, 8 * CTlo1], I16, kind="ExternalInput")
    shi1_in = nc.dram_tensor("shi1", [P, max(8 * CThi1, 16)], I16, kind="ExternalInput")
    slo2_in = nc.dram_tensor("slo2", [P, 8 * CTlo2], I16, kind="ExternalInput")
    shi2_in = nc.dram_tensor("shi2", [P, max(8 * CThi2, 16)], I16, kind="ExternalInput")
    adlo_in = nc.dram_tensor("adlo", [P, NLPAD // 16], I16, kind="ExternalInput")
    adhi_in = nc.dram_tensor("adhi", [P, NLPAD // 16], I16, kind="ExternalInput")
    mlo_in = nc.dram_tensor("mlo", [P, NW], F16, kind="ExternalInput")
    a2lo_in = nc.dram_tensor("a2lo", [P, NLPAD // 16], I16, kind="ExternalInput")
    a2hi_in = nc.dram_tensor("a2hi", [P, NLPAD // 16], I16, kind="ExternalInput")
    m2lo_in = nc.dram_tensor("m2lo", [P, NW], F16, kind="ExternalInput")
    m2hi_in = nc.dram_tensor("m2hi", [P, NW], F16, kind="ExternalInput")
    out_ext = nc.dram_tensor("out", [NLPAD, OUT], F32, kind="ExternalOutput")

    with tile.TileContext(nc) as tc:
        with (
            nc.allow_low_precision(reason="fp16 datapath; rel-err budget 2e-2"),
            tc.tile_pool(name="dram", bufs=1, space="DRAM") as dram,
            tc.tile_pool(name="const", bufs=1) as cpool,
            tc.tile_pool(name="idx", bufs=1) as ipool,
            tc.tile_pool(name="xst", bufs=2) as xpool,
            tc.tile_pool(name="hst", bufs=2) as hpool,
            tc.tile_pool(name="gbuf", bufs=2) as gpool,
            tc.tile_pool(name="g2buf", bufs=2) as g2pool,
            tc.tile_pool(name="tmp1", bufs=1) as tpool,
            tc.tile_pool(name="aux", bufs=2) as apool,
            tc.tile_pool(name="small", bufs=3) as smpool,
            tc.tile_pool(name="psA", bufs=2, space="PSUM") as psA,
            tc.tile_pool(name="psB", bufs=2, space="PSUM") as psB,
            tc.tile_pool(name="psC", bufs=2, space="PSUM") as psC,
        ):
            # ---- DRAM scratch ----
            h_tab = dram.tile([NPAD, HC1], F16)
            aDfull = dram.tile([NPAD, P], F16)   # only cols 0:8 written
            h2perm = dram.tile([NLPAD, 2 * OUT], F16)
            cc_space = "Shared" if NCORES > 1 else "Local"
            h2_tab = dram.tile([N, 2 * OUT], F16, addr_space=cc_space)

            # ---- resident constants ----
            ident = cpool.tile([P, P], F16)
            make_identity(nc, ident[:])
            asr = cpool.tile([P, HC1], F16)
            nc.sync.dma_start(out=asr[:], in_=asr_in[:])
            b1r = cpool.tile([P, HC1], F16)
            nc.sync.dma_start(out=b1r[:], in_=b1r_in[:])
            b2r = cpool.tile([P, OUT], F32)
            nc.sync.dma_start(out=b2r[:], in_=b2r_in[:])
            mlo = cpool.tile([P, NW], F16)
            nc.sync.dma_start(out=mlo[:], in_=mlo_in[:])
            m2lo = cpool.tile([P, NW], F16)
            nc.sync.dma_start(out=m2lo[:], in_=m2lo_in[:])
            m2hi = cpool.tile([P, NW], F16)
            nc.sync.dma_start(out=m2hi[:], in_=m2hi_in[:])
            adlo = cpool.tile([P, NLPAD // 16], I16)
            nc.sync.dma_start(out=adlo[:], in_=adlo_in[:])
            adhi = cpool.tile([P, NLPAD // 16], I16)
            nc.sync.dma_start(out=adhi[:], in_=adhi_in[:])
            a2lo = cpool.tile([P, NLPAD // 16], I16)
            nc.sync.dma_start(out=a2lo[:], in_=a2lo_in[:])
            a2hi = cpool.tile([P, NLPAD // 16], I16)
            nc.sync.dma_start(out=a2hi[:], in_=a2hi_in[:])
            # phase-shared (reloaded between phases 2 and 3)
            slo_t = ipool.tile([P, max(8 * CTlo1, 8 * CTlo2)], I16)
            shi_t = ipool.tile([P, max(8 * CThi1, 8 * CThi2, 16)], I16)
            mask_t = ipool.tile([P, max(CT1, CT2)], F16)
            nc.sync.dma_start(out=slo_t[:, 0:8 * CTlo1], in_=slo1_in[:])
            nc.sync.dma_start(out=shi_t[:, 0:max(8 * CThi1, 16)], in_=shi1_in[:])
            nc.sync.dma_start(out=mask_t[:, 0:CT1], in_=m1_in[:])

            # ---- phase 0: extended weights ----
            w1ext = cpool.tile([P, kt1, HC1 + H1], F16)
            for kt in range(kt1):
                nc.sync.dma_start(out=w1ext[:, kt, 0:HC1],
                                  in_=w1_in[kt * P:(kt + 1) * P, :])
            w1T_sb = smpool.tile([P, ckt, IN_DIM], F16, tag="w0")
            amat_sb = smpool.tile([P, ckt, H1], F16, tag="w0b")
            for c in range(ckt):
                nc.sync.dma_start(out=w1T_sb[:, c, :], in_=w1T_in[c * P:(c + 1) * P, :])
                nc.sync.dma_start(out=amat_sb[:, c, :], in_=amat_in[c * P:(c + 1) * P, :])
            for kt in range(kt1):
                wps = psB.tile([P, H1], F32, tag="tp0")
                for c in range(ckt):
                    nc.tensor.matmul(out=wps[:], lhsT=w1T_sb[:, c, kt * P:(kt + 1) * P],
                                     rhs=amat_sb[:, c, :], start=(c == 0), stop=(c == ckt - 1))
                nc.scalar.copy(out=w1ext[:, kt, HC1:], in_=wps[:])

            w2ext = cpool.tile([P, ckt, OUT + 2], F16)
            w2T_sb = smpool.tile([OUT, HC1], F16, tag="w0c")
            a2_sb = smpool.tile([OUT, 2], F16, tag="w0d")
            nc.sync.dma_start(out=w2T_sb[:], in_=w2T_in[:])
            nc.sync.dma_start(out=a2_sb[:], in_=a2_in[:])
            for c in range(ckt):
                nc.sync.dma_start(out=w2ext[:, c, 0:OUT], in_=w2_in[c * P:(c + 1) * P, :])
                wps2 = psB.tile([P, 2], F32, tag="tp0")
                nc.tensor.matmul(out=wps2[:], lhsT=w2T_sb[:, c * P:(c + 1) * P],
                                 rhs=a2_sb[:], start=True, stop=True)
                nc.scalar.copy(out=w2ext[:, c, OUT:], in_=wps2[:])

            # ---- phase 1: h_tab = x @ W1, aDfull = h @ Adst ----
            for g in range(_cdiv(ntiles, NB)):
                nt0 = g * NB
                nb = min(NB, ntiles - nt0)
                xst = xpool.tile([P, kt1, NB * P], F16, tag="xst")
                for kt in range(kt1):
                    nc.sync.dma_start(out=xst[:, kt, 0:nb * P],
                                      in_=xT_in[kt * P:(kt + 1) * P, nt0 * P:(nt0 + nb) * P])
                hstg = hpool.tile([P, NB, HC1 + H1], F16, tag="hst")
                for j in range(nb):
                    ps = psA.tile([P, HC1 + H1], F32, tag="mm")
                    for kt in range(kt1):
                        nc.tensor.matmul(out=ps[:], lhsT=xst[:, kt, j * P:(j + 1) * P],
                                         rhs=w1ext[:, kt, :], start=(kt == 0), stop=(kt == kt1 - 1))
                    nc.scalar.copy(out=hstg[:, j, :], in_=ps[:])
                hv = h_tab[nt0 * P:(nt0 + nb) * P, :].rearrange("(j p) c -> p j c", p=P)
                nc.sync.dma_start(out=hv, in_=hstg[:, 0:nb, 0:HC1])
                av = aDfull[nt0 * P:(nt0 + nb) * P, 0:H1].rearrange("(j p) c -> p j c", p=P)
                nc.sync.dma_start(out=av, in_=hstg[:, 0:nb, HC1:])

            def bounce_out(src_dram, cols, cast=False):
                for w in range(NW):
                    if cast:
                        t16 = smpool.tile([P, OUT], F16, tag="bz16")
                        nc.sync.dma_start(out=t16[:],
                                          in_=src_dram[w * P:(w + 1) * P, 0:cols])
                        t32 = smpool.tile([P, OUT], F32, tag="bz32")
                        nc.scalar.copy(out=t32[:], in_=t16[:])
                        nc.sync.dma_start(out=out_ext[w * P:(w + 1) * P, :], in_=t32[:])
                    else:
                        t32 = smpool.tile([P, OUT], F32, tag="bz32")
                        nc.sync.dma_start(out=t32[:],
                                          in_=src_dram[w * P:(w + 1) * P, 0:cols])
                        nc.sync.dma_start(out=out_ext[w * P:(w + 1) * P, :], in_=t32[:])

            if stop == "phase1":
                bounce_out(h_tab, OUT, cast=True)
                return nc

            # ---- aD extraction: adl_m[p, w, h] = aD of node at A-row (w,p) ----
            adl_lo = apool.tile([P, NW, P], F16, tag="aux")
            nc.gpsimd.dma_gather(out_ap=adl_lo[:], in_ap=aDfull[:],
                                 idxs_ap=adlo[:], num_idxs=NLPAD, num_idxs_reg=NLPAD,
                                 elem_size=P, single_packet=False)
            adl_hi = apool.tile([P, NW, P], F16, tag="aux")
            nc.gpsimd.dma_gather(out_ap=adl_hi[:], in_ap=aDfull[HALF:, :],
                                 idxs_ap=adhi[:], num_idxs=NLPAD, num_idxs_reg=NLPAD,
                                 elem_size=P, single_packet=False)
            adl_m = cpool.tile([P, NW, H1], F16)
            nc.vector.tensor_tensor(
                out=adl_m[:], in0=adl_lo[:, :, 0:H1],
                in1=mlo[:].unsqueeze(-1).to_broadcast((P, NW, H1)),
                op=mybir.AluOpType.mult)
            nc.vector.tensor_add(out=adl_m[:], in0=adl_m[:], in1=adl_hi[:, :, 0:H1])

            # ---- phase 2: layer-1 windows ----
            olo = oall = 0
            for w in range(NW):
                Clo, Chi, C = Clo1[w], Chi1[w], CW1[w]
                G = gpool.tile([P, cmax1, HC1], F16, tag="G")
                if Clo:
                    nc.gpsimd.dma_gather(
                        out_ap=G[:, 0:Clo, :], in_ap=h_tab[:],
                        idxs_ap=slo_t[:, 8 * olo:8 * (olo + Clo)],
                        num_idxs=Clo * P, num_idxs_reg=Clo * P, elem_size=HC1,
                        single_packet=False)
                if Chi:
                    nc.gpsimd.dma_gather(
                        out_ap=G[:, Clo:C, :], in_ap=h_tab[HALF:, :],
                        idxs_ap=shi_t[:, 8 * (oall - olo):8 * (oall - olo + Chi)],
                        num_idxs=Chi * P, num_idxs_reg=Chi * P, elem_size=HC1,
                        single_packet=False)
                # aS[e,h] = sum_j G[e, h*32+j]*a_src[h,j]  (segmented reduce)
                tmp = tpool.tile([P, cmax1, HC1], F16, tag="tmp")
                nc.vector.tensor_tensor(
                    out=tmp[:, 0:C, :], in0=G[:, 0:C, :],
                    in1=asr[:].unsqueeze(1).to_broadcast((P, C, HC1)),
                    op=mybir.AluOpType.mult)
                pex = smpool.tile([P, H1, cmax1], F16, tag="pex")
                nc.vector.tensor_reduce(
                    out=pex[:, :, 0:C],
                    in_=tmp[:, 0:C, :].rearrange("p k (h j) -> p h k j", h=H1),
                    axis=mybir.AxisListType.X, op=mybir.AluOpType.add)
                # + aD of the dst row, lrelu, exp, pad mask
                nc.vector.tensor_tensor(
                    out=pex[:, :, 0:C], in0=pex[:, :, 0:C],
                    in1=adl_m[:, w, :].unsqueeze(-1).to_broadcast((P, H1, C)),
                    op=mybir.AluOpType.add)
                nc.vector.scalar_tensor_tensor(
                    out=pex[:, :, 0:C], in0=pex[:, :, 0:C], scalar=neg,
                    in1=pex[:, :, 0:C], op0=mybir.AluOpType.mult,
                    op1=mybir.AluOpType.max)
                nc.scalar.activation(out=pex[:, :, 0:C], in_=pex[:, :, 0:C],
                                     func=mybir.ActivationFunctionType.Exp)
                nc.vector.tensor_tensor(
                    out=pex[:, :, 0:C], in0=pex[:, :, 0:C],
                    in1=mask_t[:, oall:oall + C].unsqueeze(1).to_broadcast((P, H1, C)),
                    op=mybir.AluOpType.mult)
                den = smpool.tile([P, H1], F16, tag="den")
                nc.vector.tensor_reduce(out=den[:], in_=pex[:, :, 0:C],
                                        axis=mybir.AxisListType.X,
                                        op=mybir.AluOpType.add)
                nc.vector.tensor_scalar(out=den[:], in0=den[:], scalar1=6.1e-5,
                                        scalar2=None, op0=mybir.AluOpType.max)
                rec = smpool.tile([P, H1], F16, tag="rec")
                nc.vector.reciprocal(out=rec[:], in_=den[:])
                # alpha-weight G rows, then accumulate over chunks
                nc.vector.tensor_tensor(
                    out=G[:, 0:C, :].rearrange("p k (h j) -> p k h j", h=H1),
                    in0=G[:, 0:C, :].rearrange("p k (h j) -> p k h j", h=H1),
                    in1=pex[:, :, 0:C].rearrange("p h k -> p k h").unsqueeze(-1)
                        .to_broadcast((P, C, H1, HID)),
                    op=mybir.AluOpType.mult)
                acc = smpool.tile([P, HC1], F16, tag="acc")
                if C == 1:
                    nc.scalar.copy(out=acc[:], in_=G[:, 0, :])
                else:
                    nc.vector.tensor_add(out=acc[:], in0=G[:, 0, :], in1=G[:, 1, :])
                    for k in range(2, C):
                        nc.vector.tensor_add(out=acc[:], in0=acc[:], in1=G[:, k, :])
                h1w = smpool.tile([P, HC1], F16, tag="h1w")
                nc.vector.tensor_tensor(
                    out=h1w[:].rearrange("p (h j) -> p h j", h=H1),
                    in0=acc[:].rearrange("p (h j) -> p h j", h=H1),
                    in1=rec[:].unsqueeze(-1).to_broadcast((P, H1, HID)),
                    op=mybir.AluOpType.mult)
                nc.vector.tensor_add(out=h1w[:], in0=h1w[:], in1=b1r[:])
                nc.vector.tensor_scalar(out=h1w[:], in0=h1w[:], scalar1=0.0,
                                        scalar2=None, op0=mybir.AluOpType.max)
                # h2_ext = h1 @ w2ext  -> h2perm rows (A-slot order)
                h1T = smpool.tile([P, ckt, P], F16, tag="h1T")
                for c in range(ckt):
                    tp = psB.tile([P, P], F16, tag="tp")
                    nc.tensor.transpose(tp[:], h1w[:, c * P:(c + 1) * P], ident[:])
                    nc.scalar.copy(out=h1T[:, c, :], in_=tp[:])
                h2ps = psC.tile([P, OUT + 2], F32, tag="h2")
                for c in range(ckt):
                    nc.tensor.matmul(out=h2ps[:], lhsT=h1T[:, c, :], rhs=w2ext[:, c, :],
                                     start=(c == 0), stop=(c == ckt - 1))
                h2sb = smpool.tile([P, OUT + 2], F16, tag="h2sb")
                nc.scalar.copy(out=h2sb[:], in_=h2ps[:])
                nc.sync.dma_start(out=h2perm[w * P:(w + 1) * P, 0:OUT + 2],
                                  in_=h2sb[:])
                olo += Clo
                oall += C

            if stop == "h1":
                return nc
            if stop == "phase2":
                bounce_out(h2perm, OUT, cast=True)
                return nc

            # ---- all-gather h2 (node-ordered rows) ----
            nc.gpsimd.collective_compute(
                "AllGather", mybir.AluOpType.bypass,
                replica_groups=[list(range(NCORES))],
                ins=[h2perm[0:NCHUNK, :].opt()], outs=[h2_tab[:].opt()])

            if stop == "cc":
                bounce_out(h2_tab, OUT, cast=True)
                return nc

            # ---- a2 extraction: a2w[p, w, :2] = [aS2, aD2] of node at B-row ----
            a2w_lo = apool.tile([P, NW, P], F16, tag="aux")
            nc.gpsimd.dma_gather(out_ap=a2w_lo[:], in_ap=h2_tab[:],
                                 idxs_ap=a2lo[:], num_idxs=NLPAD, num_idxs_reg=NLPAD,
                                 elem_size=P, single_packet=False)
            a2w_hi = apool.tile([P, NW, P], F16, tag="aux")
            nc.gpsimd.dma_gather(out_ap=a2w_hi[:], in_ap=h2_tab[HALF:, :],
                                 idxs_ap=a2hi[:], num_idxs=NLPAD, num_idxs_reg=NLPAD,
                                 elem_size=P, single_packet=False)
            a2w = cpool.tile([P, NW, 2], F16)
            t2a = smpool.tile([P, NW, 2], F16, tag="t2a")
            nc.vector.tensor_tensor(
                out=a2w[:], in0=a2w_lo[:, :, OUT:OUT + 2],
                in1=m2lo[:].unsqueeze(-1).to_broadcast((P, NW, 2)),
                op=mybir.AluOpType.mult)
            nc.vector.tensor_tensor(
                out=t2a[:], in0=a2w_hi[:, :, OUT:OUT + 2],
                in1=m2hi[:].unsqueeze(-1).to_broadcast((P, NW, 2)),
                op=mybir.AluOpType.mult)
            nc.vector.tensor_add(out=a2w[:], in0=a2w[:], in1=t2a[:])

            # reload idx/mask for layer 2
            nc.sync.dma_start(out=slo_t[:, 0:8 * CTlo2], in_=slo2_in[:])
            nc.sync.dma_start(out=shi_t[:, 0:max(8 * CThi2, 16)], in_=shi2_in[:])
            nc.sync.dma_start(out=mask_t[:, 0:CT2], in_=m2_in[:])

            # ---- phase 3: layer-2 windows + log_softmax ----
            t_all = cpool.tile([P, NW, OUT], F32)
            s_all = cpool.tile([P, NW], F32)
            olo = oall = 0
            for w in range(NW):
                Clo, Chi, C = Clo2[w], Chi2[w], CW2[w]
                G2 = g2pool.tile([P, cmax2, 2 * OUT], F16, tag="G2")
                if Clo:
                    nc.gpsimd.dma_gather(
                        out_ap=G2[:, 0:Clo, :], in_ap=h2_tab[:],
                        idxs_ap=slo_t[:, 8 * olo:8 * (olo + Clo)],
                        num_idxs=Clo * P, num_idxs_reg=Clo * P, elem_size=2 * OUT,
                        single_packet=False)
                if Chi:
                    nc.gpsimd.dma_gather(
                        out_ap=G2[:, Clo:C, :], in_ap=h2_tab[HALF:, :],
                        idxs_ap=shi_t[:, 8 * (oall - olo):8 * (oall - olo + Chi)],
                        num_idxs=Chi * P, num_idxs_reg=Chi * P, elem_size=2 * OUT,
                        single_packet=False)
                p2 = smpool.tile([P, cmax2], F16, tag="p2")
                nc.vector.tensor_tensor(
                    out=p2[:, 0:C], in0=G2[:, 0:C, OUT].squeeze(),
                    in1=a2w[:, w, 1].to_broadcast((P, C)),
                    op=mybir.AluOpType.add)
                nc.vector.scalar_tensor_tensor(
                    out=p2[:, 0:C], in0=p2[:, 0:C], scalar=neg, in1=p2[:, 0:C],
                    op0=mybir.AluOpType.mult, op1=mybir.AluOpType.max)
                nc.scalar.activation(out=p2[:, 0:C], in_=p2[:, 0:C],
                                     func=mybir.ActivationFunctionType.Exp)
                nc.vector.tensor_tensor(
                    out=p2[:, 0:C], in0=p2[:, 0:C], in1=mask_t[:, oall:oall + C],
                    op=mybir.AluOpType.mult)
                den2 = smpool.tile([P, 1], F16, tag="den2")
                nc.vector.tensor_reduce(out=den2[:], in_=p2[:, 0:C],
                                        axis=mybir.AxisListType.X,
                                        op=mybir.AluOpType.add)
                nc.vector.tensor_scalar(out=den2[:], in0=den2[:], scalar1=6.1e-5,
                                        scalar2=None, op0=mybir.AluOpType.max)
                rec2 = smpool.tile([P, 1], F16, tag="rec2")
                nc.vector.reciprocal(out=rec2[:], in_=den2[:])
                nc.vector.tensor_tensor(
                    out=G2[:, 0:C, 0:OUT], in0=G2[:, 0:C, 0:OUT],
                    in1=p2[:, 0:C].unsqueeze(-1).to_broadcast((P, C, OUT)),
                    op=mybir.AluOpType.mult)
                agg = smpool.tile([P, OUT], F16, tag="agg")
                if C == 1:
                    nc.scalar.copy(out=agg[:], in_=G2[:, 0, 0:OUT])
                else:
                    nc.vector.tensor_add(out=agg[:], in0=G2[:, 0, 0:OUT],
                                         in1=G2[:, 1, 0:OUT])
                    for k in range(2, C):
                        nc.vector.tensor_add(out=agg[:], in0=agg[:],
                                             in1=G2[:, k, 0:OUT])
                z = smpool.tile([P, OUT], F32, tag="z")
                nc.vector.tensor_tensor(out=z[:], in0=agg[:],
                                        in1=rec2[:].to_broadcast((P, OUT)),
                                        op=mybir.AluOpType.mult)
                nc.vector.tensor_add(out=z[:], in0=z[:], in1=b2r[:])
                negmax = smpool.tile([P, 1], F32, tag="ngm")
                nc.vector.tensor_reduce(out=negmax[:], in_=z[:], axis=mybir.AxisListType.X,
                                        op=mybir.AluOpType.max, negate=True)
                nc.vector.tensor_scalar(out=t_all[:, w, :], in0=z[:], scalar1=negmax[:],
                                        scalar2=None, op0=mybir.AluOpType.add)
                esc = smpool.tile([P, OUT], F32, tag="esc")
                nc.scalar.activation(out=esc[:], in_=t_all[:, w, :],
                                     func=mybir.ActivationFunctionType.Exp,
                                     accum_out=s_all[:, w:w + 1])
                olo += Clo
                oall += C
            # epilogue: res = t - ln(s)
            lns = cpool.tile([P, NW], F32)
            nc.scalar.activation(out=lns[:], in_=s_all[:],
                                 func=mybir.ActivationFunctionType.Ln)
            for w in range(NW):
                res = smpool.tile([P, OUT], F32, tag="res")
                nc.vector.tensor_scalar(out=res[:], in0=t_all[:, w, :], scalar1=lns[:, w:w + 1],
                                        scalar2=None, op0=mybir.AluOpType.subtract)
                nc.sync.dma_start(out=out_ext[w * P:(w + 1) * P, :], in_=res[:])

    return nc


# ----------------------------------------------------------------------------
# Host-side input packing.
# ----------------------------------------------------------------------------
def make_in_maps(inputs, cfg):
    x = np.asarray(inputs["x"], np.float32)
    ei = np.asarray(inputs["edge_index"])
    W1 = np.asarray(inputs["W1"], np.float32)
    a_src1 = np.asarray(inputs["a_src1"], np.float32)
    a_dst1 = np.asarray(inputs["a_dst1"], np.float32)
    b1 = np.asarray(inputs["b1"], np.float32)
    W2 = np.asarray(inputs["W2"], np.float32)
    a_src2 = np.asarray(inputs["a_src2"], np.float32)
    a_dst2 = np.asarray(inputs["a_dst2"], np.float32)
    b2 = np.asarray(inputs["b2"], np.float32)

    xT = np.zeros((IN_DIM, NPAD), np.float16)
    xT[:, :N] = x.T

    amat = np.zeros((HC1, H1), np.float16)
    asr_flat = np.zeros(HC1, np.float32)
    for h in range(H1):
        amat[h * HID:(h + 1) * HID, h] = a_dst1[h]
        asr_flat[h * HID:(h + 1) * HID] = a_src1[h]
    a2 = np.stack([a_src2[0], a_dst2[0]], axis=1).astype(np.float16)

    pe = prep_edges(ei)
    for k in ("Clo1", "Chi1", "Clo2", "Chi2"):
        cfg[k] = pe[k]
    cfg["cB"], cfg["qB"] = pe["cB"], pe["qB"]

    common = {
        "xT": xT,
        "W1": W1.astype(np.float16),
        "W1T": np.ascontiguousarray(W1.T).astype(np.float16),
        "Amat": amat,
        "W2": W2.astype(np.float16),
        "W2T": np.ascontiguousarray(W2.T).astype(np.float16),
        "A2": a2,
        "asr": np.tile(asr_flat[None, :], (P, 1)).astype(np.float16),
        "b1r": np.tile(b1[None, :], (P, 1)).astype(np.float16),
        "b2r": np.tile(b2[None, :], (P, 1)).astype(np.float32),
    }
    in_maps = []
    for c in range(NCORES):
        m = dict(common)
        m["mask1"] = pe["mask1"][c]
        m["mask2"] = pe["mask2"][c]
        m["slo1"] = np.ascontiguousarray(pe["slo1"][c])
        shi1 = pe["shi1"][c]
        if shi1.shape[1] < 16:
            shi1 = np.zeros((P, 16), np.int16)
        m["shi1"] = np.ascontiguousarray(shi1)
        m["slo2"] = np.ascontiguousarray(pe["slo2"][c])
        shi2 = pe["shi2"][c]
        if shi2.shape[1] < 16:
            shi2 = np.zeros((P, 16), np.int16)
        m["shi2"] = np.ascontiguousarray(shi2)
        m["adlo"] = _wrap16(pe["adlo"][c])
        m["adhi"] = _wrap16(pe["adhi"][c])
        m["mlo"] = np.ascontiguousarray(pe["mlo"][c].reshape(NW, P).T)
        m["a2lo"] = _wrap16(pe["a2lo"][c])
        m["a2hi"] = _wrap16(pe["a2hi"][c])
        m["m2lo"] = np.ascontiguousarray(pe["m2lo"][c].reshape(NW, P).T)
        m["m2hi"] = np.ascontiguousarray(pe["m2hi"][c].reshape(NW, P).T)
        in_maps.append(m)
    return in_maps


TRACE = False
LAST_RESULTS = None
STOP = ""


def kernel(**inputs) -> np.ndarray:
    global LAST_RESULTS
    from concourse.bass_utils import run_bass_kernel_spmd

    cfg = {}
    if STOP:
        cfg["STOP"] = STOP
    in_maps = make_in_maps(inputs, cfg)
    nc = build_nc(cfg)
    if not nc.is_finalized():
        nc.finalize()
    res = run_bass_kernel_spmd(nc, in_maps, core_ids=list(range(NCORES)),
                               trace=TRACE)
    LAST_RESULTS = res
    outs = np.stack([res.results[c]["out"] for c in range(NCORES)])  # [8, NLPAD, 64]
    cB, qB = cfg["cB"], cfg["qB"]
    return outs[cB, qB].astype(np.float32)


# revision 11
# speedup vs baseline: 1.4277x; 1.2756x over previous
"""GAT (2-layer, PyG-style) on 8 Trainium2 NeuronCores via Bass/Tile.

v2: dst-aligned slot layout + fp16 datapath.

  - Nodes are globally sorted by (lo_indeg//3, hi_indeg) and dealt
    round-robin to (window, row, core): window w, partition-row p of core c
    holds node order[w*1024 + p*8 + c].  All 8 cores see statistically
    identical windows, so the SPMD-uniform per-window slot counts
    (Clo[w]/Chi[w] = max per-half indegree over rows and cores) stay tight
    (~+18% padding).  Dealing also makes each core's row index q equal the
    node-order row of its chunk, so the h2 table needs no unpermute pass.
  - Edge slots are dst-row aligned: window w, chunk k, partition p holds
    the k-th in-edge of the node at row p (lo chunks [0,Clo), hi chunks
    [Clo,C)).  The per-edge gather brings the src row of h_tab (256 fp16
    = 512B).  Everything downstream is per-partition independent:
    attention logits via DVE segmented reduce, aD via a per-window
    [128,8] tile (no per-edge dst gather), segment softmax via free-dim
    reduce, aggregation via a chunk-accumulate DVE loop (no one-hot
    matmuls).
  - Layer 2 repeats the scheme with its own node ordering (the lo/hi
    split by h2-table row differs from layer 1), gathering 256B rows
    [h2(64) | aS2 | aD2 | pad] so the src attention term rides along.
  - One AllGather of h2 (node-ordered 256B rows); log_softmax in f32.
"""
import sys

for _p in ("/opt/trn_rl_repo", "/opt/pypackages"):
    if _p not in sys.path:
        sys.path.insert(0, _p)

import numpy as np
from concourse import bacc, bass, mybir, tile
from concourse.masks import make_identity

P = 128
F32 = mybir.dt.float32
F16 = mybir.dt.float16
I16 = mybir.dt.int16
HALF = 32768

# ---- problem constants (nn_GAT_60000693125135) ----
N = 50000
IN_DIM = 256
H1 = 8          # heads layer 1
HID = 32        # per-head dim layer 1
HC1 = H1 * HID  # 256
OUT = 64
NCORES = 8
NEG_SLOPE = 0.2
NCHUNK = N // NCORES            # 6250
NW = -(-NCHUNK // P)            # 49
NLPAD = NW * P                  # 6272
NPAD = -(-N // P) * P           # 50176
ZHI = N + 100 - HALF            # a zero row in aDfull's hi half


def _cdiv(a, b):
    return -(-a // b)


def _wrap16(vals):
    """int16 idx stream (len % 16 == 0) -> [128, len//16] wrap-16 layout."""
    a = np.asarray(vals, np.int16).reshape(-1, 16).T
    return np.tile(a, (8, 1))


# ----------------------------------------------------------------------------
# Host-side preprocessing.
# ----------------------------------------------------------------------------
def _plan(lo_cnt, hi_cnt, band=3):
    """Global sort by (lo//band, hi); deal to (w, p, c).

    Returns g_of_node (node -> global sorted pos), Clo[NW], Chi[NW]."""
    order = np.lexsort((hi_cnt, lo_cnt // band))
    g_of = np.empty(N, np.int64)
    g_of[order] = np.arange(N)
    lo_s = np.zeros(NW * P * NCORES, np.int64)
    hi_s = np.zeros(NW * P * NCORES, np.int64)
    lo_s[:N] = lo_cnt[order]
    hi_s[:N] = hi_cnt[order]
    Clo = lo_s.reshape(NW, P * NCORES).max(1)
    Chi = hi_s.reshape(NW, P * NCORES).max(1)
    return g_of, [int(v) for v in Clo], [int(v) for v in Chi]


def _fill_slots(src_val, dst_node, half, g_of, Clo, Chi):
    """Per-core wrapped idx streams + mask for one layer.

    Slot for the k-th (by edge order) half-edge of the node at (w, p, c):
    lo stream pos (olo[w]+k)*128+p, hi stream pos (ohi[w]+k)*128+p.
    Returns slo[nc], shi[nc], mask[nc][128, CT] (f16)."""
    CTlo, CThi = sum(Clo), sum(Chi)
    CW = [a + b for a, b in zip(Clo, Chi)]
    CT = sum(CW)
    olo = np.concatenate([[0], np.cumsum(Clo)])[:-1]
    ohi = np.concatenate([[0], np.cumsum(Chi)])[:-1]
    oall = np.concatenate([[0], np.cumsum(CW)])[:-1]

    g = g_of[dst_node]
    c = g % NCORES
    p = (g // NCORES) % P
    w = g // (P * NCORES)

    # occurrence index within (dst node, half)
    key = g * 2 + half
    sort = np.argsort(key, kind="stable")
    ks = key[sort]
    starts = np.concatenate([[0], np.cumsum(np.bincount(ks, minlength=2 * N))])
    occ = np.arange(key.size) - starts[ks]
    k_of = np.empty_like(occ)
    k_of[sort] = occ

    lo_pos = (olo[w] + k_of) * P + p
    hi_pos = (ohi[w] + k_of) * P + p
    m_pos = (oall[w] + np.where(half == 0, k_of, np.asarray(Clo)[w] + k_of)) * P + p

    slo, shi, mask = [], [], []
    for ci in range(NCORES):
        mc = c == ci
        ml = mc & (half == 0)
        mh = mc & (half == 1)
        lo_flat = np.zeros(max(CTlo, 1) * P, np.int16)
        hi_flat = np.zeros(max(CThi, 1) * P, np.int16)
        m_flat = np.zeros(CT * P, np.float16)
        lo_flat[lo_pos[ml]] = src_val[ml].astype(np.int16)
        hi_flat[hi_pos[mh]] = src_val[mh].astype(np.int16)
        m_flat[m_pos[mc]] = 1.0
        slo.append(_wrap16(lo_flat))
        shi.append(_wrap16(hi_flat))
        mask.append(np.ascontiguousarray(m_flat.reshape(CT, P).T))
    return slo, shi, mask


def prep_edges(edge_index):
    e0 = edge_index[0].astype(np.int64)
    e1 = edge_index[1].astype(np.int64)
    loops = np.arange(N, dtype=np.int64)
    src = np.concatenate([e0, loops])
    dst = np.concatenate([e1, loops])

    # ---- layer 1: split by src < HALF ----
    half1 = (src >= HALF).astype(np.int64)
    lo1 = np.bincount(dst[half1 == 0], minlength=N)
    hi1 = np.bincount(dst[half1 == 1], minlength=N)
    gA, Clo1, Chi1 = _plan(lo1, hi1)
    slo1, shi1, mask1 = _fill_slots(src - half1 * HALF, dst, half1, gA, Clo1, Chi1)

    cA = gA % NCORES
    qA = (gA // NCORES) % P + (gA // (P * NCORES)) * P   # < NCHUNK
    h2row = cA * NCHUNK + qA                             # node's h2_tab row

    # ---- layer 2: split by h2row < HALF ----
    half2 = (h2row[src] >= HALF).astype(np.int64)
    lo2 = np.bincount(dst[half2 == 0], minlength=N)
    hi2 = np.bincount(dst[half2 == 1], minlength=N)
    gB, Clo2, Chi2 = _plan(lo2, hi2)
    slo2, shi2, mask2 = _fill_slots(h2row[src] - half2 * HALF, dst, half2,
                                    gB, Clo2, Chi2)
    cB = gB % NCORES
    qB = (gB // NCORES) % P + (gB // (P * NCORES)) * P

    # ---- aux idx streams (per core, len NLPAD, trailing rows dummy) ----
    adlo = np.zeros((NCORES, NLPAD), np.int16)
    adhi = np.full((NCORES, NLPAD), ZHI, np.int16)
    mlo = np.zeros((NCORES, NLPAD), np.float16)
    a2lo = np.zeros((NCORES, NLPAD), np.int16)
    a2hi = np.zeros((NCORES, NLPAD), np.int16)
    m2lo = np.zeros((NCORES, NLPAD), np.float16)
    m2hi = np.zeros((NCORES, NLPAD), np.float16)
    nodes = np.arange(N)
    # layer-1 aD extraction: A-row q of core c holds node (cA==c, qA==q)
    is_lo = nodes < HALF
    adlo[cA[is_lo], qA[is_lo]] = nodes[is_lo].astype(np.int16)
    mlo[cA[is_lo], qA[is_lo]] = 1.0
    adhi[cA[~is_lo], qA[~is_lo]] = (nodes[~is_lo] - HALF).astype(np.int16)
    # layer-2 a2 extraction from h2_tab: B-row q of core c holds node with
    # (cB==c, qB==q); its h2_tab row is h2row[node].
    is_lo2 = h2row < HALF
    a2lo[cB[is_lo2], qB[is_lo2]] = h2row[is_lo2].astype(np.int16)
    m2lo[cB[is_lo2], qB[is_lo2]] = 1.0
    a2hi[cB[~is_lo2], qB[~is_lo2]] = (h2row[~is_lo2] - HALF).astype(np.int16)
    m2hi[cB[~is_lo2], qB[~is_lo2]] = 1.0

    return dict(
        Clo1=Clo1, Chi1=Chi1, Clo2=Clo2, Chi2=Chi2,
        slo1=slo1, shi1=shi1, mask1=mask1,
        slo2=slo2, shi2=shi2, mask2=mask2,
        adlo=adlo, adhi=adhi, mlo=mlo,
        a2lo=a2lo, a2hi=a2hi, m2lo=m2lo, m2hi=m2hi,
        cB=cB, qB=qB,
    )


# ----------------------------------------------------------------------------
# Kernel builder (SPMD program, same for all cores).
# ----------------------------------------------------------------------------
def build_nc(cfg):
    neg = NEG_SLOPE
    Clo1, Chi1 = cfg["Clo1"], cfg["Chi1"]
    Clo2, Chi2 = cfg["Clo2"], cfg["Chi2"]
    CW1 = [a + b for a, b in zip(Clo1, Chi1)]
    CW2 = [a + b for a, b in zip(Clo2, Chi2)]
    CT1, CT2 = sum(CW1), sum(CW2)
    CTlo1, CThi1 = sum(Clo1), sum(Chi1)
    CTlo2, CThi2 = sum(Clo2), sum(Chi2)
    cmax1, cmax2 = max(CW1), max(CW2)
    ntiles = NPAD // P
    NB = 8
    kt1 = IN_DIM // P     # 2
    ckt = HC1 // P        # 2
    stop = cfg.get("STOP", "")

    nc = bacc.Bacc(None, target_bir_lowering=False, debug=False,
                   num_devices=NCORES, num_swdge_queues=4)
    _qn = [0]

    def qn():
        _qn[0] = (_qn[0] + 1) % 4
        return _qn[0]

    # ---- I/O ----
    xT_in = nc.dram_tensor("xT", [IN_DIM, NPAD], F16, kind="ExternalInput")
    w1_in = nc.dram_tensor("W1", [IN_DIM, HC1], F16, kind="ExternalInput")
    w1T_in = nc.dram_tensor("W1T", [HC1, IN_DIM], F16, kind="ExternalInput")
    amat_in = nc.dram_tensor("Amat", [HC1, H1], F16, kind="ExternalInput")
    w2_in = nc.dram_tensor("W2", [HC1, OUT], F16, kind="ExternalInput")
    w2T_in = nc.dram_tensor("W2T", [OUT, HC1], F16, kind="ExternalInput")
    a2_in = nc.dram_tensor("A2", [OUT, 2], F16, kind="ExternalInput")
    asr_in = nc.dram_tensor("asr", [P, HC1], F16, kind="ExternalInput")
    b1r_in = nc.dram_tensor("b1r", [P, HC1], F16, kind="ExternalInput")
    b2r_in = nc.dram_tensor("b2r", [P, OUT], F32, kind="ExternalInput")
    m1_in = nc.dram_tensor("mask1", [P, CT1], F16, kind="ExternalInput")
    m2_in = nc.dram_tensor("mask2", [P, CT2], F16, kind="ExternalInput")
    slo1_in = nc.dram_tensor("slo1", [P, 8 * CTlo1], I16, kind="ExternalInput")
    shi1_in = nc.dram_tensor("shi1", [P, max(8 * CThi1, 16)], I16, kind="ExternalInput")
    slo2_in = nc.dram_tensor("slo2", [P, 8 * CTlo2], I16, kind="ExternalInput")
    shi2_in = nc.dram_tensor("shi2", [P, max(8 * CThi2, 16)], I16, kind="ExternalInput")
    adlo_in = nc.dram_tensor("adlo", [P, NLPAD // 16], I16, kind="ExternalInput")
    adhi_in = nc.dram_tensor("adhi", [P, NLPAD // 16], I16, kind="ExternalInput")
    mlo_in = nc.dram_tensor("mlo", [P, NW], F16, kind="ExternalInput")
    a2lo_in = nc.dram_tensor("a2lo", [P, NLPAD // 16], I16, kind="ExternalInput")
    a2hi_in = nc.dram_tensor("a2hi", [P, NLPAD // 16], I16, kind="ExternalInput")
    m2lo_in = nc.dram_tensor("m2lo", [P, NW], F16, kind="ExternalInput")
    m2hi_in = nc.dram_tensor("m2hi", [P, NW], F16, kind="ExternalInput")
    out_ext = nc.dram_tensor("out", [NLPAD, OUT], F32, kind="ExternalOutput")

    with tile.TileContext(nc) as tc:
        with (
            nc.allow_low_precision(reason="fp16 datapath; rel-err budget 2e-2"),
            tc.tile_pool(name="dram", bufs=1, space="DRAM") as dram,
            tc.tile_pool(name="const", bufs=1) as cpool,
            tc.tile_pool(name="idx", bufs=1) as ipool,
            tc.tile_pool(name="xst", bufs=2) as xpool,
            tc.tile_pool(name="hst", bufs=2) as hpool,
            tc.tile_pool(name="gbuf", bufs=2) as gpool,
            tc.tile_pool(name="g2buf", bufs=2) as g2pool,
            tc.tile_pool(name="tmp1", bufs=1) as tpool,
            tc.tile_pool(name="aux", bufs=2) as apool,
            tc.tile_pool(name="small", bufs=3) as smpool,
            tc.tile_pool(name="psA", bufs=2, space="PSUM") as psA,
            tc.tile_pool(name="psB", bufs=2, space="PSUM") as psB,
            tc.tile_pool(name="psC", bufs=2, space="PSUM") as psC,
        ):
            # ---- DRAM scratch ----
            h_tab = dram.tile([NPAD, HC1], F16)
            aDfull = dram.tile([NPAD, P], F16)   # only cols 0:8 written
            h2perm = dram.tile([NLPAD, 2 * OUT], F16)
            cc_space = "Shared" if NCORES > 1 else "Local"
            h2_tab = dram.tile([N, 2 * OUT], F16, addr_space=cc_space)

            # ---- resident constants ----
            ident = cpool.tile([P, P], F16)
            make_identity(nc, ident[:])
            asr = cpool.tile([P, HC1], F16)
            nc.sync.dma_start(out=asr[:], in_=asr_in[:])
            b1r = cpool.tile([P, HC1], F16)
            nc.sync.dma_start(out=b1r[:], in_=b1r_in[:])
            b2r = cpool.tile([P, OUT], F32)
            nc.sync.dma_start(out=b2r[:], in_=b2r_in[:])
            mlo = cpool.tile([P, NW], F16)
            nc.sync.dma_start(out=mlo[:], in_=mlo_in[:])
            m2lo = cpool.tile([P, NW], F16)
            nc.sync.dma_start(out=m2lo[:], in_=m2lo_in[:])
            m2hi = cpool.tile([P, NW], F16)
            nc.sync.dma_start(out=m2hi[:], in_=m2hi_in[:])
            adlo = cpool.tile([P, NLPAD // 16], I16)
            nc.sync.dma_start(out=adlo[:], in_=adlo_in[:])
            adhi = cpool.tile([P, NLPAD // 16], I16)
            nc.sync.dma_start(out=adhi[:], in_=adhi_in[:])
            a2lo = cpool.tile([P, NLPAD // 16], I16)
            nc.sync.dma_start(out=a2lo[:], in_=a2lo_in[:])
            a2hi = cpool.tile([P, NLPAD // 16], I16)
            nc.sync.dma_start(out=a2hi[:], in_=a2hi_in[:])
            # phase-shared (reloaded between phases 2 and 3)
            slo_t = ipool.tile([P, max(8 * CTlo1, 8 * CTlo2)], I16)
            shi_t = ipool.tile([P, max(8 * CThi1, 8 * CThi2, 16)], I16)
            mask_t = ipool.tile([P, max(CT1, CT2)], F16)
            nc.sync.dma_start(out=slo_t[:, 0:8 * CTlo1], in_=slo1_in[:])
            nc.sync.dma_start(out=shi_t[:, 0:max(8 * CThi1, 16)], in_=shi1_in[:])
            nc.sync.dma_start(out=mask_t[:, 0:CT1], in_=m1_in[:])

            # ---- phase 0: extended weights ----
            w1ext = cpool.tile([P, kt1, HC1 + H1], F16)
            for kt in range(kt1):
                nc.sync.dma_start(out=w1ext[:, kt, 0:HC1],
                                  in_=w1_in[kt * P:(kt + 1) * P, :])
            w1T_sb = smpool.tile([P, ckt, IN_DIM], F16, tag="w0")
            amat_sb = smpool.tile([P, ckt, H1], F16, tag="w0b")
            for c in range(ckt):
                nc.sync.dma_start(out=w1T_sb[:, c, :], in_=w1T_in[c * P:(c + 1) * P, :])
                nc.sync.dma_start(out=amat_sb[:, c, :], in_=amat_in[c * P:(c + 1) * P, :])
            for kt in range(kt1):
                wps = psB.tile([P, H1], F32, tag="tp0")
                for c in range(ckt):
                    nc.tensor.matmul(out=wps[:], lhsT=w1T_sb[:, c, kt * P:(kt + 1) * P],
                                     rhs=amat_sb[:, c, :], start=(c == 0), stop=(c == ckt - 1))
                nc.scalar.copy(out=w1ext[:, kt, HC1:], in_=wps[:])

            w2ext = cpool.tile([P, ckt, OUT + 2], F16)
            w2T_sb = smpool.tile([OUT, HC1], F16, tag="w0c")
            a2_sb = smpool.tile([OUT, 2], F16, tag="w0d")
            nc.sync.dma_start(out=w2T_sb[:], in_=w2T_in[:])
            nc.sync.dma_start(out=a2_sb[:], in_=a2_in[:])
            for c in range(ckt):
                nc.sync.dma_start(out=w2ext[:, c, 0:OUT], in_=w2_in[c * P:(c + 1) * P, :])
                wps2 = psB.tile([P, 2], F32, tag="tp0")
                nc.tensor.matmul(out=wps2[:], lhsT=w2T_sb[:, c * P:(c + 1) * P],
                                 rhs=a2_sb[:], start=True, stop=True)
                nc.scalar.copy(out=w2ext[:, c, OUT:], in_=wps2[:])

            # ---- phase 1: h_tab = x @ W1, aDfull = h @ Adst ----
            for g in range(_cdiv(ntiles, NB)):
                nt0 = g * NB
                nb = min(NB, ntiles - nt0)
                xst = xpool.tile([P, kt1, NB * P], F16, tag="xst")
                for kt in range(kt1):
                    nc.sync.dma_start(out=xst[:, kt, 0:nb * P],
                                      in_=xT_in[kt * P:(kt + 1) * P, nt0 * P:(nt0 + nb) * P])
                hstg = hpool.tile([P, NB, HC1 + H1], F16, tag="hst")
                for j in range(nb):
                    ps = psA.tile([P, HC1 + H1], F32, tag="mm")
                    for kt in range(kt1):
                        nc.tensor.matmul(out=ps[:], lhsT=xst[:, kt, j * P:(j + 1) * P],
                                         rhs=w1ext[:, kt, :], start=(kt == 0), stop=(kt == kt1 - 1))
                    nc.scalar.copy(out=hstg[:, j, :], in_=ps[:])
                hv = h_tab[nt0 * P:(nt0 + nb) * P, :].rearrange("(j p) c -> p j c", p=P)
                nc.sync.dma_start(out=hv, in_=hstg[:, 0:nb, 0:HC1])
                av = aDfull[nt0 * P:(nt0 + nb) * P, 0:H1].rearrange("(j p) c -> p j c", p=P)
                nc.sync.dma_start(out=av, in_=hstg[:, 0:nb, HC1:])

            def bounce_out(src_dram, cols, cast=False):
                for w in range(NW):
                    if cast:
                        t16 = smpool.tile([P, OUT], F16, tag="bz16")
                        nc.sync.dma_start(out=t16[:],
                                          in_=src_dram[w * P:(w + 1) * P, 0:cols])
                        t32 = smpool.tile([P, OUT], F32, tag="bz32")
                        nc.scalar.copy(out=t32[:], in_=t16[:])
                        nc.sync.dma_start(out=out_ext[w * P:(w + 1) * P, :], in_=t32[:])
                    else:
                        t32 = smpool.tile([P, OUT], F32, tag="bz32")
                        nc.sync.dma_start(out=t32[:],
                                          in_=src_dram[w * P:(w + 1) * P, 0:cols])
                        nc.sync.dma_start(out=out_ext[w * P:(w + 1) * P, :], in_=t32[:])

            if stop == "phase1":
                bounce_out(h_tab, OUT, cast=True)
                return nc

            # ---- aD extraction: adl_m[p, w, h] = aD of node at A-row (w,p) ----
            adl_lo = apool.tile([P, NW, P], F16, tag="aux")
            nc.gpsimd.dma_gather(out_ap=adl_lo[:], in_ap=aDfull[:],
                                 idxs_ap=adlo[:], num_idxs=NLPAD, num_idxs_reg=NLPAD,
                                 elem_size=P, single_packet=False, queue_num=qn())
            adl_hi = apool.tile([P, NW, P], F16, tag="aux")
            nc.gpsimd.dma_gather(out_ap=adl_hi[:], in_ap=aDfull[HALF:, :],
                                 idxs_ap=adhi[:], num_idxs=NLPAD, num_idxs_reg=NLPAD,
                                 elem_size=P, single_packet=False, queue_num=qn())
            adl_m = cpool.tile([P, NW, H1], F16)
            nc.vector.tensor_tensor(
                out=adl_m[:], in0=adl_lo[:, :, 0:H1],
                in1=mlo[:].unsqueeze(-1).to_broadcast((P, NW, H1)),
                op=mybir.AluOpType.mult)
            nc.vector.tensor_add(out=adl_m[:], in0=adl_m[:], in1=adl_hi[:, :, 0:H1])

            # ---- phase 2: layer-1 windows ----
            olo = oall = 0
            for w in range(NW):
                Clo, Chi, C = Clo1[w], Chi1[w], CW1[w]
                G = gpool.tile([P, cmax1, HC1], F16, tag="G")
                if Clo:
                    nc.gpsimd.dma_gather(
                        out_ap=G[:, 0:Clo, :], in_ap=h_tab[:],
                        idxs_ap=slo_t[:, 8 * olo:8 * (olo + Clo)],
                        num_idxs=Clo * P, num_idxs_reg=Clo * P, elem_size=HC1,
                        single_packet=False, queue_num=qn())
                if Chi:
                    nc.gpsimd.dma_gather(
                        out_ap=G[:, Clo:C, :], in_ap=h_tab[HALF:, :],
                        idxs_ap=shi_t[:, 8 * (oall - olo):8 * (oall - olo + Chi)],
                        num_idxs=Chi * P, num_idxs_reg=Chi * P, elem_size=HC1,
                        single_packet=False, queue_num=qn())
                # aS[e,h] = sum_j G[e, h*32+j]*a_src[h,j]  (segmented reduce)
                tmp = tpool.tile([P, cmax1, HC1], F16, tag="tmp")
                nc.vector.tensor_tensor(
                    out=tmp[:, 0:C, :], in0=G[:, 0:C, :],
                    in1=asr[:].unsqueeze(1).to_broadcast((P, C, HC1)),
                    op=mybir.AluOpType.mult)
                pex = smpool.tile([P, H1, cmax1], F16, tag="pex")
                nc.vector.tensor_reduce(
                    out=pex[:, :, 0:C],
                    in_=tmp[:, 0:C, :].rearrange("p k (h j) -> p h k j", h=H1),
                    axis=mybir.AxisListType.X, op=mybir.AluOpType.add)
                # + aD of the dst row, lrelu, exp, pad mask
                nc.vector.tensor_tensor(
                    out=pex[:, :, 0:C], in0=pex[:, :, 0:C],
                    in1=adl_m[:, w, :].unsqueeze(-1).to_broadcast((P, H1, C)),
                    op=mybir.AluOpType.add)
                nc.vector.scalar_tensor_tensor(
                    out=pex[:, :, 0:C], in0=pex[:, :, 0:C], scalar=neg,
                    in1=pex[:, :, 0:C], op0=mybir.AluOpType.mult,
                    op1=mybir.AluOpType.max)
                nc.scalar.activation(out=pex[:, :, 0:C], in_=pex[:, :, 0:C],
                                     func=mybir.ActivationFunctionType.Exp)
                nc.vector.tensor_tensor(
                    out=pex[:, :, 0:C], in0=pex[:, :, 0:C],
                    in1=mask_t[:, oall:oall + C].unsqueeze(1).to_broadcast((P, H1, C)),
                    op=mybir.AluOpType.mult)
                den = smpool.tile([P, H1], F16, tag="den")
                nc.vector.tensor_reduce(out=den[:], in_=pex[:, :, 0:C],
                                        axis=mybir.AxisListType.X,
                                        op=mybir.AluOpType.add)
                nc.vector.tensor_scalar(out=den[:], in0=den[:], scalar1=6.1e-5,
                                        scalar2=None, op0=mybir.AluOpType.max)
                rec = smpool.tile([P, H1], F16, tag="rec")
                nc.vector.reciprocal(out=rec[:], in_=den[:])
                # alpha-weight G rows, then accumulate over chunks
                nc.vector.tensor_tensor(
                    out=G[:, 0:C, :].rearrange("p k (h j) -> p k h j", h=H1),
                    in0=G[:, 0:C, :].rearrange("p k (h j) -> p k h j", h=H1),
                    in1=pex[:, :, 0:C].rearrange("p h k -> p k h").unsqueeze(-1)
                        .to_broadcast((P, C, H1, HID)),
                    op=mybir.AluOpType.mult)
                acc = smpool.tile([P, HC1], F16, tag="acc")
                if C == 1:
                    nc.scalar.copy(out=acc[:], in_=G[:, 0, :])
                else:
                    nc.vector.tensor_add(out=acc[:], in0=G[:, 0, :], in1=G[:, 1, :])
                    for k in range(2, C):
                        nc.vector.tensor_add(out=acc[:], in0=acc[:], in1=G[:, k, :])
                h1w = smpool.tile([P, HC1], F16, tag="h1w")
                nc.vector.tensor_tensor(
                    out=h1w[:].rearrange("p (h j) -> p h j", h=H1),
                    in0=acc[:].rearrange("p (h j) -> p h j", h=H1),
                    in1=rec[:].unsqueeze(-1).to_broadcast((P, H1, HID)),
                    op=mybir.AluOpType.mult)
                nc.vector.tensor_add(out=h1w[:], in0=h1w[:], in1=b1r[:])
                nc.vector.tensor_scalar(out=h1w[:], in0=h1w[:], scalar1=0.0,
                                        scalar2=None, op0=mybir.AluOpType.max)
                # h2_ext = h1 @ w2ext  -> h2perm rows (A-slot order)
                h1T = smpool.tile([P, ckt, P], F16, tag="h1T")
                for c in range(ckt):
                    tp = psB.tile([P, P], F16, tag="tp")
                    nc.tensor.transpose(tp[:], h1w[:, c * P:(c + 1) * P], ident[:])
                    nc.scalar.copy(out=h1T[:, c, :], in_=tp[:])
                h2ps = psC.tile([P, OUT + 2], F32, tag="h2")
                for c in range(ckt):
                    nc.tensor.matmul(out=h2ps[:], lhsT=h1T[:, c, :], rhs=w2ext[:, c, :],
                                     start=(c == 0), stop=(c == ckt - 1))
                h2sb = smpool.tile([P, OUT + 2], F16, tag="h2sb")
                nc.scalar.copy(out=h2sb[:], in_=h2ps[:])
                nc.sync.dma_start(out=h2perm[w * P:(w + 1) * P, 0:OUT + 2],
                                  in_=h2sb[:])
                olo += Clo
                oall += C

            if stop == "h1":
                return nc
            if stop == "phase2":
                bounce_out(h2perm, OUT, cast=True)
                return nc

            # ---- all-gather h2 (node-ordered rows) ----
            nc.gpsimd.collective_compute(
                "AllGather", mybir.AluOpType.bypass,
                replica_groups=[list(range(NCORES))],
                ins=[h2perm[0:NCHUNK, :].opt()], outs=[h2_tab[:].opt()])

            if stop == "cc":
                bounce_out(h2_tab, OUT, cast=True)
                return nc

            # ---- a2 extraction: a2w[p, w, :2] = [aS2, aD2] of node at B-row ----
            a2w_lo = apool.tile([P, NW, P], F16, tag="aux")
            nc.gpsimd.dma_gather(out_ap=a2w_lo[:], in_ap=h2_tab[:],
                                 idxs_ap=a2lo[:], num_idxs=NLPAD, num_idxs_reg=NLPAD,
                                 elem_size=P, single_packet=False, queue_num=qn())
            a2w_hi = apool.tile([P, NW, P], F16, tag="aux")
            nc.gpsimd.dma_gather(out_ap=a2w_hi[:], in_ap=h2_tab[HALF:, :],
                                 idxs_ap=a2hi[:], num_idxs=NLPAD, num_idxs_reg=NLPAD,
                                 elem_size=P, single_packet=False, queue_num=qn())
            a2w = cpool.tile([P, NW, 2], F16)
            t2a = smpool.tile([P, NW, 2], F16, tag="t2a")
            nc.vector.tensor_tensor(
                out=a2w[:], in0=a2w_lo[:, :, OUT:OUT + 2],
                in1=m2lo[:].unsqueeze(-1).to_broadcast((P, NW, 2)),
                op=mybir.AluOpType.mult)
            nc.vector.tensor_tensor(
                out=t2a[:], in0=a2w_hi[:, :, OUT:OUT + 2],
                in1=m2hi[:].unsqueeze(-1).to_broadcast((P, NW, 2)),
                op=mybir.AluOpType.mult)
            nc.vector.tensor_add(out=a2w[:], in0=a2w[:], in1=t2a[:])

            # reload idx/mask for layer 2
            nc.sync.dma_start(out=slo_t[:, 0:8 * CTlo2], in_=slo2_in[:])
            nc.sync.dma_start(out=shi_t[:, 0:max(8 * CThi2, 16)], in_=shi2_in[:])
            nc.sync.dma_start(out=mask_t[:, 0:CT2], in_=m2_in[:])

            # ---- phase 3: layer-2 windows + log_softmax ----
            t_all = cpool.tile([P, NW, OUT], F32)
            s_all = cpool.tile([P, NW], F32)
            olo = oall = 0
            for w in range(NW):
                Clo, Chi, C = Clo2[w], Chi2[w], CW2[w]
                G2 = g2pool.tile([P, cmax2, 2 * OUT], F16, tag="G2")
                if Clo:
                    nc.gpsimd.dma_gather(
                        out_ap=G2[:, 0:Clo, :], in_ap=h2_tab[:],
                        idxs_ap=slo_t[:, 8 * olo:8 * (olo + Clo)],
                        num_idxs=Clo * P, num_idxs_reg=Clo * P, elem_size=2 * OUT,
                        single_packet=False, queue_num=qn())
                if Chi:
                    nc.gpsimd.dma_gather(
                        out_ap=G2[:, Clo:C, :], in_ap=h2_tab[HALF:, :],
                        idxs_ap=shi_t[:, 8 * (oall - olo):8 * (oall - olo + Chi)],
                        num_idxs=Chi * P, num_idxs_reg=Chi * P, elem_size=2 * OUT,
                        single_packet=False, queue_num=qn())
                p2 = smpool.tile([P, cmax2], F16, tag="p2")
                nc.vector.tensor_tensor(
                    out=p2[:, 0:C], in0=G2[:, 0:C, OUT].squeeze(),
                    in1=a2w[:, w, 1].to_broadcast((P, C)),
                    op=mybir.AluOpType.add)
                nc.vector.scalar_tensor_tensor(
                    out=p2[:, 0:C], in0=p2[:, 0:C], scalar=neg, in1=p2[:, 0:C],
                    op0=mybir.AluOpType.mult, op1=mybir.AluOpType.max)
                nc.scalar.activation(out=p2[:, 0:C], in_=p2[:, 0:C],
                                     func=mybir.ActivationFunctionType.Exp)
                nc.vector.tensor_tensor(
                    out=p2[:, 0:C], in0=p2[:, 0:C], in1=mask_t[:, oall:oall + C],
                    op=mybir.AluOpType.mult)
                den2 = smpool.tile([P, 1], F16, tag="den2")
                nc.vector.tensor_reduce(out=den2[:], in_=p2[:, 0:C],
                                        axis=mybir.AxisListType.X,
                                        op=mybir.AluOpType.add)
                nc.vector.tensor_scalar(out=den2[:], in0=den2[:], scalar1=6.1e-5,
                                        scalar2=None, op0=mybir.AluOpType.max)
                rec2 = smpool.tile([P, 1], F16, tag="rec2")
                nc.vector.reciprocal(out=rec2[:], in_=den2[:])
                nc.vector.tensor_tensor(
                    out=G2[:, 0:C, 0:OUT], in0=G2[:, 0:C, 0:OUT],
                    in1=p2[:, 0:C].unsqueeze(-1).to_broadcast((P, C, OUT)),
                    op=mybir.AluOpType.mult)
                agg = smpool.tile([P, OUT], F16, tag="agg")
                if C == 1:
                    nc.scalar.copy(out=agg[:], in_=G2[:, 0, 0:OUT])
                else:
                    nc.vector.tensor_add(out=agg[:], in0=G2[:, 0, 0:OUT],
                                         in1=G2[:, 1, 0:OUT])
                    for k in range(2, C):
                        nc.vector.tensor_add(out=agg[:], in0=agg[:],
                                             in1=G2[:, k, 0:OUT])
                z = smpool.tile([P, OUT], F32, tag="z")
                nc.vector.tensor_tensor(out=z[:], in0=agg[:],
                                        in1=rec2[:].to_broadcast((P, OUT)),
                                        op=mybir.AluOpType.mult)
                nc.vector.tensor_add(out=z[:], in0=z[:], in1=b2r[:])
                negmax = smpool.tile([P, 1], F32, tag="ngm")
                nc.vector.tensor_reduce(out=negmax[:], in_=z[:], axis=mybir.AxisListType.X,
                                        op=mybir.AluOpType.max, negate=True)
                nc.vector.tensor_scalar(out=t_all[:, w, :], in0=z[:], scalar1=negmax[:],
                                        scalar2=None, op0=mybir.AluOpType.add)
                esc = smpool.tile([P, OUT], F32, tag="esc")
                nc.scalar.activation(out=esc[:], in_=t_all[:, w, :],
                                     func=mybir.ActivationFunctionType.Exp,
                                     accum_out=s_all[:, w:w + 1])
                olo += Clo
                oall += C
            # epilogue: res = t - ln(s)
            lns = cpool.tile([P, NW], F32)
            nc.scalar.activation(out=lns[:], in_=s_all[:],
                                 func=mybir.ActivationFunctionType.Ln)
            for w in range(NW):
                res = smpool.tile([P, OUT], F32, tag="res")
                nc.vector.tensor_scalar(out=res[:], in0=t_all[:, w, :], scalar1=lns[:, w:w + 1],
                                        scalar2=None, op0=mybir.AluOpType.subtract)
                nc.sync.dma_start(out=out_ext[w * P:(w + 1) * P, :], in_=res[:])

    return nc


# ----------------------------------------------------------------------------
# Host-side input packing.
# ----------------------------------------------------------------------------
def make_in_maps(inputs, cfg):
    x = np.asarray(inputs["x"], np.float32)
    ei = np.asarray(inputs["edge_index"])
    W1 = np.asarray(inputs["W1"], np.float32)
    a_src1 = np.asarray(inputs["a_src1"], np.float32)
    a_dst1 = np.asarray(inputs["a_dst1"], np.float32)
    b1 = np.asarray(inputs["b1"], np.float32)
    W2 = np.asarray(inputs["W2"], np.float32)
    a_src2 = np.asarray(inputs["a_src2"], np.float32)
    a_dst2 = np.asarray(inputs["a_dst2"], np.float32)
    b2 = np.asarray(inputs["b2"], np.float32)

    xT = np.zeros((IN_DIM, NPAD), np.float16)
    xT[:, :N] = x.T

    amat = np.zeros((HC1, H1), np.float16)
    asr_flat = np.zeros(HC1, np.float32)
    for h in range(H1):
        amat[h * HID:(h + 1) * HID, h] = a_dst1[h]
        asr_flat[h * HID:(h + 1) * HID] = a_src1[h]
    a2 = np.stack([a_src2[0], a_dst2[0]], axis=1).astype(np.float16)

    pe = prep_edges(ei)
    for k in ("Clo1", "Chi1", "Clo2", "Chi2"):
        cfg[k] = pe[k]
    cfg["cB"], cfg["qB"] = pe["cB"], pe["qB"]

    common = {
        "xT": xT,
        "W1": W1.astype(np.float16),
        "W1T": np.ascontiguousarray(W1.T).astype(np.float16),
        "Amat": amat,
        "W2": W2.astype(np.float16),
        "W2T": np.ascontiguousarray(W2.T).astype(np.float16),
        "A2": a2,
        "asr": np.tile(asr_flat[None, :], (P, 1)).astype(np.float16),
        "b1r": np.tile(b1[None, :], (P, 1)).astype(np.float16),
        "b2r": np.tile(b2[None, :], (P, 1)).astype(np.float32),
    }
    in_maps = []
    for c in range(NCORES):
        m = dict(common)
        m["mask1"] = pe["mask1"][c]
        m["mask2"] = pe["mask2"][c]
        m["slo1"] = np.ascontiguousarray(pe["slo1"][c])
        shi1 = pe["shi1"][c]
        if shi1.shape[1] < 16:
            shi1 = np.zeros((P, 16), np.int16)
        m["shi1"] = np.ascontiguousarray(shi1)
        m["slo2"] = np.ascontiguousarray(pe["slo2"][c])
        shi2 = pe["shi2"][c]
        if shi2.shape[1] < 16:
            shi2 = np.zeros((P, 16), np.int16)
        m["shi2"] = np.ascontiguousarray(shi2)
        m["adlo"] = _wrap16(pe["adlo"][c])
        m["adhi"] = _wrap16(pe["adhi"][c])
        m["mlo"] = np.ascontiguousarray(pe["mlo"][c].reshape(NW, P).T)
        m["a2lo"] = _wrap16(pe["a2lo"][c])
        m["a2hi"] = _wrap16(pe["a2hi"][c])
        m["m2lo"] = np.ascontiguousarray(pe["m2lo"][c].reshape(NW, P).T)
        m["m2hi"] = np.ascontiguousarray(pe["m2hi"][c].reshape(NW, P).T)
        in_maps.append(m)
    return in_maps


TRACE = False
LAST_RESULTS = None
STOP = ""


def kernel(**inputs) -> np.ndarray:
    global LAST_RESULTS
    from concourse.bass_utils import run_bass_kernel_spmd

    cfg = {}
    if STOP:
        cfg["STOP"] = STOP
    in_maps = make_in_maps(inputs, cfg)
    nc = build_nc(cfg)
    if not nc.is_finalized():
        nc.finalize()
    res = run_bass_kernel_spmd(nc, in_maps, core_ids=list(range(NCORES)),
                               trace=TRACE)
    LAST_RESULTS = res
    outs = np.stack([res.results[c]["out"] for c in range(NCORES)])  # [8, NLPAD, 64]
    cB, qB = cfg["cB"], cfg["qB"]
    return outs[cB, qB].astype(np.float32)


# revision 12
# speedup vs baseline: 1.7382x; 1.2175x over previous
"""GAT (2-layer, PyG-style) on 8 Trainium2 NeuronCores via Bass/Tile.

v2: dst-aligned slot layout + fp16 datapath.

  - Nodes are globally sorted by (lo_indeg//3, hi_indeg) and dealt
    round-robin to (window, row, core): window w, partition-row p of core c
    holds node order[w*1024 + p*8 + c].  All 8 cores see statistically
    identical windows, so the SPMD-uniform per-window slot counts
    (Clo[w]/Chi[w] = max per-half indegree over rows and cores) stay tight
    (~+18% padding).  Dealing also makes each core's row index q equal the
    node-order row of its chunk, so the h2 table needs no unpermute pass.
  - Edge slots are dst-row aligned: window w, chunk k, partition p holds
    the k-th in-edge of the node at row p (lo chunks [0,Clo), hi chunks
    [Clo,C)).  The per-edge gather brings the src row of h_tab (256 fp16
    = 512B).  Everything downstream is per-partition independent:
    attention logits via DVE segmented reduce, aD via a per-window
    [128,8] tile (no per-edge dst gather), segment softmax via free-dim
    reduce, aggregation via a chunk-accumulate DVE loop (no one-hot
    matmuls).
  - Layer 2 repeats the scheme with its own node ordering (the lo/hi
    split by h2-table row differs from layer 1), gathering 256B rows
    [h2(64) | aS2 | aD2 | pad] so the src attention term rides along.
  - One AllGather of h2 (node-ordered 256B rows); log_softmax in f32.
"""
import sys

for _p in ("/opt/trn_rl_repo", "/opt/pypackages"):
    if _p not in sys.path:
        sys.path.insert(0, _p)

import numpy as np
from concourse import bacc, bass, mybir, tile
from concourse.masks import make_identity

P = 128
F32 = mybir.dt.float32
F16 = mybir.dt.float16
I16 = mybir.dt.int16
HALF = 32768

# ---- problem constants (nn_GAT_60000693125135) ----
N = 50000
IN_DIM = 256
H1 = 8          # heads layer 1
HID = 32        # per-head dim layer 1
HC1 = H1 * HID  # 256
OUT = 64
NCORES = 8
NEG_SLOPE = 0.2
NCHUNK = N // NCORES            # 6250
NW = -(-NCHUNK // P)            # 49
NLPAD = NW * P                  # 6272
NPAD = -(-N // P) * P           # 50176
ZHI = N + 100 - HALF            # a zero row in aDfull's hi half


def _cdiv(a, b):
    return -(-a // b)


def _wrap16(vals):
    """int16 idx stream (len % 16 == 0) -> [128, len//16] wrap-16 layout."""
    a = np.asarray(vals, np.int16).reshape(-1, 16).T
    return np.tile(a, (8, 1))


# ----------------------------------------------------------------------------
# Host-side preprocessing.
# ----------------------------------------------------------------------------
def _plan(lo_cnt, hi_cnt, band=3):
    """Global sort by (lo//band, hi); deal to (w, p, c).

    Returns g_of_node (node -> global sorted pos), Clo[NW], Chi[NW]."""
    order = np.lexsort((hi_cnt, lo_cnt // band))
    g_of = np.empty(N, np.int64)
    g_of[order] = np.arange(N)
    lo_s = np.zeros(NW * P * NCORES, np.int64)
    hi_s = np.zeros(NW * P * NCORES, np.int64)
    lo_s[:N] = lo_cnt[order]
    hi_s[:N] = hi_cnt[order]
    Clo = lo_s.reshape(NW, P * NCORES).max(1)
    Chi = hi_s.reshape(NW, P * NCORES).max(1)
    return g_of, [int(v) for v in Clo], [int(v) for v in Chi]


def _fill_slots(src_val, dst_node, half, g_of, Clo, Chi):
    """Per-core wrapped idx streams + mask for one layer.

    Slot for the k-th (by edge order) half-edge of the node at (w, p, c):
    lo stream pos (olo[w]+k)*128+p, hi stream pos (ohi[w]+k)*128+p.
    Returns slo[nc], shi[nc], mask[nc][128, CT] (f16)."""
    CTlo, CThi = sum(Clo), sum(Chi)
    CW = [a + b for a, b in zip(Clo, Chi)]
    CT = sum(CW)
    olo = np.concatenate([[0], np.cumsum(Clo)])[:-1]
    ohi = np.concatenate([[0], np.cumsum(Chi)])[:-1]
    oall = np.concatenate([[0], np.cumsum(CW)])[:-1]

    g = g_of[dst_node]
    c = g % NCORES
    p = (g // NCORES) % P
    w = g // (P * NCORES)

    # occurrence index within (dst node, half)
    key = g * 2 + half
    sort = np.argsort(key, kind="stable")
    ks = key[sort]
    starts = np.concatenate([[0], np.cumsum(np.bincount(ks, minlength=2 * N))])
    occ = np.arange(key.size) - starts[ks]
    k_of = np.empty_like(occ)
    k_of[sort] = occ

    lo_pos = (olo[w] + k_of) * P + p
    hi_pos = (ohi[w] + k_of) * P + p
    m_pos = (oall[w] + np.where(half == 0, k_of, np.asarray(Clo)[w] + k_of)) * P + p

    slo, shi, mask = [], [], []
    for ci in range(NCORES):
        mc = c == ci
        ml = mc & (half == 0)
        mh = mc & (half == 1)
        lo_flat = np.zeros(max(CTlo, 1) * P, np.int16)
        hi_flat = np.zeros(max(CThi, 1) * P, np.int16)
        m_flat = np.zeros(CT * P, np.float16)
        lo_flat[lo_pos[ml]] = src_val[ml].astype(np.int16)
        hi_flat[hi_pos[mh]] = src_val[mh].astype(np.int16)
        m_flat[m_pos[mc]] = 1.0
        slo.append(_wrap16(lo_flat))
        shi.append(_wrap16(hi_flat))
        mask.append(np.ascontiguousarray(m_flat.reshape(CT, P).T))
    return slo, shi, mask


def prep_edges(edge_index):
    e0 = edge_index[0].astype(np.int64)
    e1 = edge_index[1].astype(np.int64)
    loops = np.arange(N, dtype=np.int64)
    src = np.concatenate([e0, loops])
    dst = np.concatenate([e1, loops])

    # ---- layer 1: split by src < HALF ----
    half1 = (src >= HALF).astype(np.int64)
    lo1 = np.bincount(dst[half1 == 0], minlength=N)
    hi1 = np.bincount(dst[half1 == 1], minlength=N)
    gA, Clo1, Chi1 = _plan(lo1, hi1)
    slo1, shi1, mask1 = _fill_slots(src - half1 * HALF, dst, half1, gA, Clo1, Chi1)

    cA = gA % NCORES
    qA = (gA // NCORES) % P + (gA // (P * NCORES)) * P   # < NCHUNK
    h2row = cA * NCHUNK + qA                             # node's h2_tab row

    # ---- layer 2: split by h2row < HALF ----
    half2 = (h2row[src] >= HALF).astype(np.int64)
    lo2 = np.bincount(dst[half2 == 0], minlength=N)
    hi2 = np.bincount(dst[half2 == 1], minlength=N)
    gB, Clo2, Chi2 = _plan(lo2, hi2)
    slo2, shi2, mask2 = _fill_slots(h2row[src] - half2 * HALF, dst, half2,
                                    gB, Clo2, Chi2)
    cB = gB % NCORES
    qB = (gB // NCORES) % P + (gB // (P * NCORES)) * P

    # ---- aux idx streams (per core, len NLPAD, trailing rows dummy) ----
    adlo = np.zeros((NCORES, NLPAD), np.int16)
    adhi = np.full((NCORES, NLPAD), ZHI, np.int16)
    mlo = np.zeros((NCORES, NLPAD), np.float16)
    a2lo = np.zeros((NCORES, NLPAD), np.int16)
    a2hi = np.zeros((NCORES, NLPAD), np.int16)
    m2lo = np.zeros((NCORES, NLPAD), np.float16)
    m2hi = np.zeros((NCORES, NLPAD), np.float16)
    nodes = np.arange(N)
    # layer-1 aD extraction: A-row q of core c holds node (cA==c, qA==q)
    is_lo = nodes < HALF
    adlo[cA[is_lo], qA[is_lo]] = nodes[is_lo].astype(np.int16)
    mlo[cA[is_lo], qA[is_lo]] = 1.0
    adhi[cA[~is_lo], qA[~is_lo]] = (nodes[~is_lo] - HALF).astype(np.int16)
    # layer-2 a2 extraction from h2_tab: B-row q of core c holds node with
    # (cB==c, qB==q); its h2_tab row is h2row[node].
    is_lo2 = h2row < HALF
    a2lo[cB[is_lo2], qB[is_lo2]] = h2row[is_lo2].astype(np.int16)
    m2lo[cB[is_lo2], qB[is_lo2]] = 1.0
    a2hi[cB[~is_lo2], qB[~is_lo2]] = (h2row[~is_lo2] - HALF).astype(np.int16)
    m2hi[cB[~is_lo2], qB[~is_lo2]] = 1.0

    return dict(
        Clo1=Clo1, Chi1=Chi1, Clo2=Clo2, Chi2=Chi2,
        slo1=slo1, shi1=shi1, mask1=mask1,
        slo2=slo2, shi2=shi2, mask2=mask2,
        adlo=adlo, adhi=adhi, mlo=mlo,
        a2lo=a2lo, a2hi=a2hi, m2lo=m2lo, m2hi=m2hi,
        cB=cB, qB=qB,
    )


# ----------------------------------------------------------------------------
# Kernel builder (SPMD program, same for all cores).
# ----------------------------------------------------------------------------
def build_nc(cfg):
    neg = NEG_SLOPE
    Clo1, Chi1 = cfg["Clo1"], cfg["Chi1"]
    Clo2, Chi2 = cfg["Clo2"], cfg["Chi2"]
    CW1 = [a + b for a, b in zip(Clo1, Chi1)]
    CW2 = [a + b for a, b in zip(Clo2, Chi2)]
    CT1, CT2 = sum(CW1), sum(CW2)
    CTlo1, CThi1 = sum(Clo1), sum(Chi1)
    CTlo2, CThi2 = sum(Clo2), sum(Chi2)
    cmax1, cmax2 = max(CW1), max(CW2)
    ntiles = NPAD // P
    NB = 8
    kt1 = IN_DIM // P     # 2
    ckt = HC1 // P        # 2
    stop = cfg.get("STOP", "")

    nc = bacc.Bacc(None, target_bir_lowering=False, debug=False,
                   num_devices=NCORES, num_swdge_queues=4)
    _qn = [0]

    def qn():
        _qn[0] = (_qn[0] + 1) % 4
        return _qn[0]

    # ---- I/O ----
    xT_in = nc.dram_tensor("xT", [IN_DIM, NPAD], F16, kind="ExternalInput")
    w1_in = nc.dram_tensor("W1", [IN_DIM, HC1], F16, kind="ExternalInput")
    w1T_in = nc.dram_tensor("W1T", [HC1, IN_DIM], F16, kind="ExternalInput")
    amat_in = nc.dram_tensor("Amat", [HC1, H1], F16, kind="ExternalInput")
    w2_in = nc.dram_tensor("W2", [HC1, OUT], F16, kind="ExternalInput")
    w2T_in = nc.dram_tensor("W2T", [OUT, HC1], F16, kind="ExternalInput")
    a2_in = nc.dram_tensor("A2", [OUT, 2], F16, kind="ExternalInput")
    asr_in = nc.dram_tensor("asr", [P, HC1], F16, kind="ExternalInput")
    b1r_in = nc.dram_tensor("b1r", [P, HC1], F16, kind="ExternalInput")
    b2r_in = nc.dram_tensor("b2r", [P, OUT], F32, kind="ExternalInput")
    m1_in = nc.dram_tensor("mask1", [P, CT1], F16, kind="ExternalInput")
    m2_in = nc.dram_tensor("mask2", [P, CT2], F16, kind="ExternalInput")
    slo1_in = nc.dram_tensor("slo1", [P, 8 * CTlo1], I16, kind="ExternalInput")
    shi1_in = nc.dram_tensor("shi1", [P, max(8 * CThi1, 16)], I16, kind="ExternalInput")
    slo2_in = nc.dram_tensor("slo2", [P, 8 * CTlo2], I16, kind="ExternalInput")
    shi2_in = nc.dram_tensor("shi2", [P, max(8 * CThi2, 16)], I16, kind="ExternalInput")
    adlo_in = nc.dram_tensor("adlo", [P, NLPAD // 16], I16, kind="ExternalInput")
    adhi_in = nc.dram_tensor("adhi", [P, NLPAD // 16], I16, kind="ExternalInput")
    mlo_in = nc.dram_tensor("mlo", [P, NW], F16, kind="ExternalInput")
    a2lo_in = nc.dram_tensor("a2lo", [P, NLPAD // 16], I16, kind="ExternalInput")
    a2hi_in = nc.dram_tensor("a2hi", [P, NLPAD // 16], I16, kind="ExternalInput")
    m2lo_in = nc.dram_tensor("m2lo", [P, NW], F16, kind="ExternalInput")
    m2hi_in = nc.dram_tensor("m2hi", [P, NW], F16, kind="ExternalInput")
    out_ext = nc.dram_tensor("out", [NLPAD, OUT], F32, kind="ExternalOutput")

    with tile.TileContext(nc) as tc:
        with (
            nc.allow_low_precision(reason="fp16 datapath; rel-err budget 2e-2"),
            tc.tile_pool(name="dram", bufs=1, space="DRAM") as dram,
            tc.tile_pool(name="const", bufs=1) as cpool,
            tc.tile_pool(name="idx", bufs=1) as ipool,
            tc.tile_pool(name="xst", bufs=2) as xpool,
            tc.tile_pool(name="hst", bufs=2) as hpool,
            tc.tile_pool(name="gbuf", bufs=2) as gpool,
            tc.tile_pool(name="g2buf", bufs=2) as g2pool,
            tc.tile_pool(name="tmp1", bufs=1) as tpool,
            tc.tile_pool(name="aux", bufs=2) as apool,
            tc.tile_pool(name="small", bufs=3) as smpool,
            tc.tile_pool(name="psA", bufs=2, space="PSUM") as psA,
            tc.tile_pool(name="psB", bufs=2, space="PSUM") as psB,
            tc.tile_pool(name="psC", bufs=2, space="PSUM") as psC,
        ):
            # ---- DRAM scratch ----
            h_tab = dram.tile([NPAD, HC1], F16)
            aDfull = dram.tile([NPAD, P], F16)   # only cols 0:8 written
            h2perm = dram.tile([NLPAD, 2 * OUT], F16)
            cc_space = "Shared" if NCORES > 1 else "Local"
            h2_tab = dram.tile([N, 2 * OUT], F16, addr_space=cc_space)

            # ---- resident constants ----
            ident = cpool.tile([P, P], F16)
            make_identity(nc, ident[:])
            asr = cpool.tile([P, HC1], F16)
            nc.sync.dma_start(out=asr[:], in_=asr_in[:])
            b1r = cpool.tile([P, HC1], F16)
            nc.sync.dma_start(out=b1r[:], in_=b1r_in[:])
            b2r = cpool.tile([P, OUT], F32)
            nc.sync.dma_start(out=b2r[:], in_=b2r_in[:])
            mlo = cpool.tile([P, NW], F16)
            nc.sync.dma_start(out=mlo[:], in_=mlo_in[:])
            m2lo = cpool.tile([P, NW], F16)
            nc.sync.dma_start(out=m2lo[:], in_=m2lo_in[:])
            m2hi = cpool.tile([P, NW], F16)
            nc.sync.dma_start(out=m2hi[:], in_=m2hi_in[:])
            adlo = cpool.tile([P, NLPAD // 16], I16)
            nc.sync.dma_start(out=adlo[:], in_=adlo_in[:])
            adhi = cpool.tile([P, NLPAD // 16], I16)
            nc.sync.dma_start(out=adhi[:], in_=adhi_in[:])
            a2lo = cpool.tile([P, NLPAD // 16], I16)
            nc.sync.dma_start(out=a2lo[:], in_=a2lo_in[:])
            a2hi = cpool.tile([P, NLPAD // 16], I16)
            nc.sync.dma_start(out=a2hi[:], in_=a2hi_in[:])
            # phase-shared (reloaded between phases 2 and 3)
            slo_t = ipool.tile([P, max(8 * CTlo1, 8 * CTlo2)], I16)
            shi_t = ipool.tile([P, max(8 * CThi1, 8 * CThi2, 16)], I16)
            mask_t = ipool.tile([P, max(CT1, CT2)], F16)
            nc.sync.dma_start(out=slo_t[:, 0:8 * CTlo1], in_=slo1_in[:])
            nc.sync.dma_start(out=shi_t[:, 0:max(8 * CThi1, 16)], in_=shi1_in[:])
            nc.sync.dma_start(out=mask_t[:, 0:CT1], in_=m1_in[:])

            # ---- phase 0: extended weights ----
            w1ext = cpool.tile([P, kt1, HC1 + H1], F16)
            for kt in range(kt1):
                nc.sync.dma_start(out=w1ext[:, kt, 0:HC1],
                                  in_=w1_in[kt * P:(kt + 1) * P, :])
            w1T_sb = smpool.tile([P, ckt, IN_DIM], F16, tag="w0")
            amat_sb = smpool.tile([P, ckt, H1], F16, tag="w0b")
            for c in range(ckt):
                nc.sync.dma_start(out=w1T_sb[:, c, :], in_=w1T_in[c * P:(c + 1) * P, :])
                nc.sync.dma_start(out=amat_sb[:, c, :], in_=amat_in[c * P:(c + 1) * P, :])
            for kt in range(kt1):
                wps = psB.tile([P, H1], F32, tag="tp0")
                for c in range(ckt):
                    nc.tensor.matmul(out=wps[:], lhsT=w1T_sb[:, c, kt * P:(kt + 1) * P],
                                     rhs=amat_sb[:, c, :], start=(c == 0), stop=(c == ckt - 1))
                nc.scalar.copy(out=w1ext[:, kt, HC1:], in_=wps[:])

            w2ext = cpool.tile([P, ckt, OUT + 2], F16)
            w2T_sb = smpool.tile([OUT, HC1], F16, tag="w0c")
            a2_sb = smpool.tile([OUT, 2], F16, tag="w0d")
            nc.sync.dma_start(out=w2T_sb[:], in_=w2T_in[:])
            nc.sync.dma_start(out=a2_sb[:], in_=a2_in[:])
            for c in range(ckt):
                nc.sync.dma_start(out=w2ext[:, c, 0:OUT], in_=w2_in[c * P:(c + 1) * P, :])
                wps2 = psB.tile([P, 2], F32, tag="tp0")
                nc.tensor.matmul(out=wps2[:], lhsT=w2T_sb[:, c * P:(c + 1) * P],
                                 rhs=a2_sb[:], start=True, stop=True)
                nc.scalar.copy(out=w2ext[:, c, OUT:], in_=wps2[:])

            # ---- phase 1: h_tab = x @ W1, aDfull = h @ Adst ----
            for g in range(_cdiv(ntiles, NB)):
                nt0 = g * NB
                nb = min(NB, ntiles - nt0)
                xst = xpool.tile([P, kt1, NB * P], F16, tag="xst")
                for kt in range(kt1):
                    nc.sync.dma_start(out=xst[:, kt, 0:nb * P],
                                      in_=xT_in[kt * P:(kt + 1) * P, nt0 * P:(nt0 + nb) * P])
                hstg = hpool.tile([P, NB, HC1 + H1], F16, tag="hst")
                for j in range(nb):
                    ps = psA.tile([P, HC1 + H1], F32, tag="mm")
                    for kt in range(kt1):
                        nc.tensor.matmul(out=ps[:], lhsT=xst[:, kt, j * P:(j + 1) * P],
                                         rhs=w1ext[:, kt, :], start=(kt == 0), stop=(kt == kt1 - 1))
                    nc.scalar.copy(out=hstg[:, j, :], in_=ps[:])
                hv = h_tab[nt0 * P:(nt0 + nb) * P, :].rearrange("(j p) c -> p j c", p=P)
                nc.sync.dma_start(out=hv, in_=hstg[:, 0:nb, 0:HC1])
                av = aDfull[nt0 * P:(nt0 + nb) * P, 0:H1].rearrange("(j p) c -> p j c", p=P)
                nc.sync.dma_start(out=av, in_=hstg[:, 0:nb, HC1:])

            def bounce_out(src_dram, cols, cast=False):
                for w in range(NW):
                    if cast:
                        t16 = smpool.tile([P, OUT], F16, tag="bz16")
                        nc.sync.dma_start(out=t16[:],
                                          in_=src_dram[w * P:(w + 1) * P, 0:cols])
                        t32 = smpool.tile([P, OUT], F32, tag="bz32")
                        nc.scalar.copy(out=t32[:], in_=t16[:])
                        nc.sync.dma_start(out=out_ext[w * P:(w + 1) * P, :], in_=t32[:])
                    else:
                        t32 = smpool.tile([P, OUT], F32, tag="bz32")
                        nc.sync.dma_start(out=t32[:],
                                          in_=src_dram[w * P:(w + 1) * P, 0:cols])
                        nc.sync.dma_start(out=out_ext[w * P:(w + 1) * P, :], in_=t32[:])

            if stop == "phase1":
                bounce_out(h_tab, OUT, cast=True)
                return nc

            # ---- aD extraction: adl_m[p, w, h] = aD of node at A-row (w,p) ----
            adl_lo = apool.tile([P, NW, P], F16, tag="aux")
            nc.gpsimd.dma_gather(out_ap=adl_lo[:], in_ap=aDfull[:],
                                 idxs_ap=adlo[:], num_idxs=NLPAD, num_idxs_reg=NLPAD,
                                 elem_size=P, single_packet=False, queue_num=qn())
            adl_hi = apool.tile([P, NW, P], F16, tag="aux")
            nc.gpsimd.dma_gather(out_ap=adl_hi[:], in_ap=aDfull[HALF:, :],
                                 idxs_ap=adhi[:], num_idxs=NLPAD, num_idxs_reg=NLPAD,
                                 elem_size=P, single_packet=False, queue_num=qn())
            adl_m = cpool.tile([P, NW, H1], F16)
            nc.vector.tensor_tensor(
                out=adl_m[:], in0=adl_lo[:, :, 0:H1],
                in1=mlo[:].unsqueeze(-1).to_broadcast((P, NW, H1)),
                op=mybir.AluOpType.mult)
            nc.vector.tensor_add(out=adl_m[:], in0=adl_m[:], in1=adl_hi[:, :, 0:H1])

            # ---- phase 2: layer-1 windows ----
            olo = oall = 0
            for w in range(NW):
                Clo, Chi, C = Clo1[w], Chi1[w], CW1[w]
                G = gpool.tile([P, cmax1, HC1], F16, tag="G")
                if Clo:
                    nc.gpsimd.dma_gather(
                        out_ap=G[:, 0:Clo, :], in_ap=h_tab[:],
                        idxs_ap=slo_t[:, 8 * olo:8 * (olo + Clo)],
                        num_idxs=Clo * P, num_idxs_reg=Clo * P, elem_size=HC1,
                        single_packet=False, queue_num=qn())
                if Chi:
                    nc.gpsimd.dma_gather(
                        out_ap=G[:, Clo:C, :], in_ap=h_tab[HALF:, :],
                        idxs_ap=shi_t[:, 8 * (oall - olo):8 * (oall - olo + Chi)],
                        num_idxs=Chi * P, num_idxs_reg=Chi * P, elem_size=HC1,
                        single_packet=False, queue_num=qn())
                # aS[e,h] = sum_j G[e, h*32+j]*a_src[h,j]  (segmented reduce)
                tmp = tpool.tile([P, cmax1, HC1], F16, tag="tmp")
                nc.vector.tensor_tensor(
                    out=tmp[:, 0:C, :], in0=G[:, 0:C, :],
                    in1=asr[:].unsqueeze(1).to_broadcast((P, C, HC1)),
                    op=mybir.AluOpType.mult)
                pex = smpool.tile([P, H1, cmax1], F16, tag="pex")
                nc.vector.tensor_reduce(
                    out=pex[:, :, 0:C],
                    in_=tmp[:, 0:C, :].rearrange("p k (h j) -> p h k j", h=H1),
                    axis=mybir.AxisListType.X, op=mybir.AluOpType.add)
                # + aD of the dst row, lrelu, exp, pad mask
                nc.vector.tensor_tensor(
                    out=pex[:, :, 0:C], in0=pex[:, :, 0:C],
                    in1=adl_m[:, w, :].unsqueeze(-1).to_broadcast((P, H1, C)),
                    op=mybir.AluOpType.add)
                nc.vector.scalar_tensor_tensor(
                    out=pex[:, :, 0:C], in0=pex[:, :, 0:C], scalar=neg,
                    in1=pex[:, :, 0:C], op0=mybir.AluOpType.mult,
                    op1=mybir.AluOpType.max)
                nc.scalar.activation(out=pex[:, :, 0:C], in_=pex[:, :, 0:C],
                                     func=mybir.ActivationFunctionType.Exp)
                nc.vector.tensor_tensor(
                    out=pex[:, :, 0:C], in0=pex[:, :, 0:C],
                    in1=mask_t[:, oall:oall + C].unsqueeze(1).to_broadcast((P, H1, C)),
                    op=mybir.AluOpType.mult)
                den = smpool.tile([P, H1], F16, tag="den")
                nc.vector.tensor_reduce(out=den[:], in_=pex[:, :, 0:C],
                                        axis=mybir.AxisListType.X,
                                        op=mybir.AluOpType.add)
                rec = smpool.tile([P, H1], F16, tag="rec")
                nc.vector.reciprocal(out=rec[:], in_=den[:])
                # alpha-weight G rows, then accumulate over chunks
                nc.vector.tensor_tensor(
                    out=G[:, 0:C, :].rearrange("p k (h j) -> p k h j", h=H1),
                    in0=G[:, 0:C, :].rearrange("p k (h j) -> p k h j", h=H1),
                    in1=pex[:, :, 0:C].rearrange("p h k -> p k h").unsqueeze(-1)
                        .to_broadcast((P, C, H1, HID)),
                    op=mybir.AluOpType.mult)
                # tree-accumulate chunks: G[0:C] -> tmp halves -> acc
                acc = smpool.tile([P, HC1], F16, tag="acc")
                if C == 1:
                    nc.scalar.copy(out=acc[:], in_=G[:, 0, :])
                else:
                    n = C // 2
                    pv = G[:, 0:2 * n, :].rearrange("p (k t) c -> p k t c", t=2)
                    nc.vector.tensor_tensor(out=tmp[:, 0:n, :], in0=pv[:, :, 0, :],
                                            in1=pv[:, :, 1, :], op=mybir.AluOpType.add)
                    if C % 2:
                        nc.vector.tensor_add(out=tmp[:, 0, :], in0=tmp[:, 0, :],
                                             in1=G[:, C - 1, :])
                    while n > 1:
                        m = n // 2
                        pv = tmp[:, 0:2 * m, :].rearrange("p (k t) c -> p k t c", t=2)
                        nc.vector.tensor_tensor(out=tmp[:, 0:m, :], in0=pv[:, :, 0, :],
                                                in1=pv[:, :, 1, :], op=mybir.AluOpType.add)
                        if n % 2:
                            nc.vector.tensor_add(out=tmp[:, 0, :], in0=tmp[:, 0, :],
                                                 in1=tmp[:, n - 1, :])
                        n = m
                    nc.scalar.copy(out=acc[:], in_=tmp[:, 0, :])
                h1w = smpool.tile([P, HC1], F16, tag="h1w")
                nc.vector.tensor_tensor(
                    out=h1w[:].rearrange("p (h j) -> p h j", h=H1),
                    in0=acc[:].rearrange("p (h j) -> p h j", h=H1),
                    in1=rec[:].unsqueeze(-1).to_broadcast((P, H1, HID)),
                    op=mybir.AluOpType.mult)
                nc.vector.tensor_add(out=h1w[:], in0=h1w[:], in1=b1r[:])
                nc.scalar.activation(out=h1w[:], in_=h1w[:],
                                     func=mybir.ActivationFunctionType.Relu)
                # h2_ext = h1 @ w2ext  -> h2perm rows (A-slot order)
                h1T = smpool.tile([P, ckt, P], F16, tag="h1T")
                for c in range(ckt):
                    tp = psB.tile([P, P], F16, tag="tp")
                    nc.tensor.transpose(tp[:], h1w[:, c * P:(c + 1) * P], ident[:])
                    nc.scalar.copy(out=h1T[:, c, :], in_=tp[:])
                h2ps = psC.tile([P, OUT + 2], F32, tag="h2")
                for c in range(ckt):
                    nc.tensor.matmul(out=h2ps[:], lhsT=h1T[:, c, :], rhs=w2ext[:, c, :],
                                     start=(c == 0), stop=(c == ckt - 1))
                h2sb = smpool.tile([P, OUT + 2], F16, tag="h2sb")
                nc.scalar.copy(out=h2sb[:], in_=h2ps[:])
                nc.sync.dma_start(out=h2perm[w * P:(w + 1) * P, 0:OUT + 2],
                                  in_=h2sb[:])
                olo += Clo
                oall += C

            if stop == "h1":
                return nc
            if stop == "phase2":
                bounce_out(h2perm, OUT, cast=True)
                return nc

            # ---- all-gather h2 (node-ordered rows) ----
            nc.gpsimd.collective_compute(
                "AllGather", mybir.AluOpType.bypass,
                replica_groups=[list(range(NCORES))],
                ins=[h2perm[0:NCHUNK, :].opt()], outs=[h2_tab[:].opt()])

            if stop == "cc":
                bounce_out(h2_tab, OUT, cast=True)
                return nc

            # ---- a2 extraction: a2w[p, w, :2] = [aS2, aD2] of node at B-row ----
            a2w_lo = apool.tile([P, NW, P], F16, tag="aux")
            nc.gpsimd.dma_gather(out_ap=a2w_lo[:], in_ap=h2_tab[:],
                                 idxs_ap=a2lo[:], num_idxs=NLPAD, num_idxs_reg=NLPAD,
                                 elem_size=P, single_packet=False, queue_num=qn())
            a2w_hi = apool.tile([P, NW, P], F16, tag="aux")
            nc.gpsimd.dma_gather(out_ap=a2w_hi[:], in_ap=h2_tab[HALF:, :],
                                 idxs_ap=a2hi[:], num_idxs=NLPAD, num_idxs_reg=NLPAD,
                                 elem_size=P, single_packet=False, queue_num=qn())
            a2w = cpool.tile([P, NW, 2], F16)
            t2a = smpool.tile([P, NW, 2], F16, tag="t2a")
            nc.vector.tensor_tensor(
                out=a2w[:], in0=a2w_lo[:, :, OUT:OUT + 2],
                in1=m2lo[:].unsqueeze(-1).to_broadcast((P, NW, 2)),
                op=mybir.AluOpType.mult)
            nc.vector.tensor_tensor(
                out=t2a[:], in0=a2w_hi[:, :, OUT:OUT + 2],
                in1=m2hi[:].unsqueeze(-1).to_broadcast((P, NW, 2)),
                op=mybir.AluOpType.mult)
            nc.vector.tensor_add(out=a2w[:], in0=a2w[:], in1=t2a[:])

            # reload idx/mask for layer 2
            nc.sync.dma_start(out=slo_t[:, 0:8 * CTlo2], in_=slo2_in[:])
            nc.sync.dma_start(out=shi_t[:, 0:max(8 * CThi2, 16)], in_=shi2_in[:])
            nc.sync.dma_start(out=mask_t[:, 0:CT2], in_=m2_in[:])

            # ---- phase 3: layer-2 windows + log_softmax ----
            t_all = cpool.tile([P, NW, OUT], F32)
            s_all = cpool.tile([P, NW], F32)
            olo = oall = 0
            for w in range(NW):
                Clo, Chi, C = Clo2[w], Chi2[w], CW2[w]
                G2 = g2pool.tile([P, cmax2, 2 * OUT], F16, tag="G2")
                if Clo:
                    nc.gpsimd.dma_gather(
                        out_ap=G2[:, 0:Clo, :], in_ap=h2_tab[:],
                        idxs_ap=slo_t[:, 8 * olo:8 * (olo + Clo)],
                        num_idxs=Clo * P, num_idxs_reg=Clo * P, elem_size=2 * OUT,
                        single_packet=False, queue_num=qn())
                if Chi:
                    nc.gpsimd.dma_gather(
                        out_ap=G2[:, Clo:C, :], in_ap=h2_tab[HALF:, :],
                        idxs_ap=shi_t[:, 8 * (oall - olo):8 * (oall - olo + Chi)],
                        num_idxs=Chi * P, num_idxs_reg=Chi * P, elem_size=2 * OUT,
                        single_packet=False, queue_num=qn())
                p2 = smpool.tile([P, cmax2], F16, tag="p2")
                nc.vector.tensor_tensor(
                    out=p2[:, 0:C], in0=G2[:, 0:C, OUT].squeeze(),
                    in1=a2w[:, w, 1].to_broadcast((P, C)),
                    op=mybir.AluOpType.add)
                nc.vector.scalar_tensor_tensor(
                    out=p2[:, 0:C], in0=p2[:, 0:C], scalar=neg, in1=p2[:, 0:C],
                    op0=mybir.AluOpType.mult, op1=mybir.AluOpType.max)
                nc.scalar.activation(out=p2[:, 0:C], in_=p2[:, 0:C],
                                     func=mybir.ActivationFunctionType.Exp)
                nc.vector.tensor_tensor(
                    out=p2[:, 0:C], in0=p2[:, 0:C], in1=mask_t[:, oall:oall + C],
                    op=mybir.AluOpType.mult)
                den2 = smpool.tile([P, 1], F16, tag="den2")
                nc.vector.tensor_reduce(out=den2[:], in_=p2[:, 0:C],
                                        axis=mybir.AxisListType.X,
                                        op=mybir.AluOpType.add)
                rec2 = smpool.tile([P, 1], F16, tag="rec2")
                nc.vector.reciprocal(out=rec2[:], in_=den2[:])
                nc.vector.tensor_tensor(
                    out=G2[:, 0:C, 0:OUT], in0=G2[:, 0:C, 0:OUT],
                    in1=p2[:, 0:C].unsqueeze(-1).to_broadcast((P, C, OUT)),
                    op=mybir.AluOpType.mult)
                agg = smpool.tile([P, OUT], F16, tag="agg")
                t3 = smpool.tile([P, cmax2 // 2 + 1, OUT], F16, tag="t3")
                if C == 1:
                    nc.scalar.copy(out=agg[:], in_=G2[:, 0, 0:OUT])
                else:
                    n = C // 2
                    pv = G2[:, 0:2 * n, :].rearrange("p (k t) c -> p k t c", t=2)
                    nc.vector.tensor_tensor(out=t3[:, 0:n, :], in0=pv[:, :, 0, 0:OUT],
                                            in1=pv[:, :, 1, 0:OUT], op=mybir.AluOpType.add)
                    if C % 2:
                        nc.vector.tensor_add(out=t3[:, 0, :], in0=t3[:, 0, :],
                                             in1=G2[:, C - 1, 0:OUT])
                    while n > 1:
                        m = n // 2
                        pv = t3[:, 0:2 * m, :].rearrange("p (k t) c -> p k t c", t=2)
                        nc.vector.tensor_tensor(out=t3[:, 0:m, :], in0=pv[:, :, 0, :],
                                                in1=pv[:, :, 1, :], op=mybir.AluOpType.add)
                        if n % 2:
                            nc.vector.tensor_add(out=t3[:, 0, :], in0=t3[:, 0, :],
                                                 in1=t3[:, n - 1, :])
                        n = m
                    nc.scalar.copy(out=agg[:], in_=t3[:, 0, :])
                z = smpool.tile([P, OUT], F32, tag="z")
                nc.vector.tensor_tensor(out=z[:], in0=agg[:],
                                        in1=rec2[:].to_broadcast((P, OUT)),
                                        op=mybir.AluOpType.mult)
                nc.vector.tensor_add(out=z[:], in0=z[:], in1=b2r[:])
                negmax = smpool.tile([P, 1], F32, tag="ngm")
                nc.vector.tensor_reduce(out=negmax[:], in_=z[:], axis=mybir.AxisListType.X,
                                        op=mybir.AluOpType.max, negate=True)
                nc.vector.tensor_tensor(out=t_all[:, w, :], in0=z[:],
                                        in1=negmax[:].to_broadcast((P, OUT)),
                                        op=mybir.AluOpType.add)
                esc = smpool.tile([P, OUT], F32, tag="esc")
                nc.scalar.activation(out=esc[:], in_=t_all[:, w, :],
                                     func=mybir.ActivationFunctionType.Exp,
                                     accum_out=s_all[:, w:w + 1])
                olo += Clo
                oall += C
            # epilogue: res = t - ln(s)
            lns = cpool.tile([P, NW], F32)
            nc.scalar.activation(out=lns[:], in_=s_all[:],
                                 func=mybir.ActivationFunctionType.Ln)
            for w in range(NW):
                res = smpool.tile([P, OUT], F32, tag="res")
                nc.vector.tensor_tensor(out=res[:], in0=t_all[:, w, :],
                                        in1=lns[:, w:w + 1].to_broadcast((P, OUT)),
                                        op=mybir.AluOpType.subtract)
                nc.sync.dma_start(out=out_ext[w * P:(w + 1) * P, :], in_=res[:])

    return nc


# ----------------------------------------------------------------------------
# Host-side input packing.
# ----------------------------------------------------------------------------
def make_in_maps(inputs, cfg):
    x = np.asarray(inputs["x"], np.float32)
    ei = np.asarray(inputs["edge_index"])
    W1 = np.asarray(inputs["W1"], np.float32)
    a_src1 = np.asarray(inputs["a_src1"], np.float32)
    a_dst1 = np.asarray(inputs["a_dst1"], np.float32)
    b1 = np.asarray(inputs["b1"], np.float32)
    W2 = np.asarray(inputs["W2"], np.float32)
    a_src2 = np.asarray(inputs["a_src2"], np.float32)
    a_dst2 = np.asarray(inputs["a_dst2"], np.float32)
    b2 = np.asarray(inputs["b2"], np.float32)

    xT = np.zeros((IN_DIM, NPAD), np.float16)
    xT[:, :N] = x.T

    amat = np.zeros((HC1, H1), np.float16)
    asr_flat = np.zeros(HC1, np.float32)
    for h in range(H1):
        amat[h * HID:(h + 1) * HID, h] = a_dst1[h]
        asr_flat[h * HID:(h + 1) * HID] = a_src1[h]
    a2 = np.stack([a_src2[0], a_dst2[0]], axis=1).astype(np.float16)

    pe = prep_edges(ei)
    for k in ("Clo1", "Chi1", "Clo2", "Chi2"):
        cfg[k] = pe[k]
    cfg["cB"], cfg["qB"] = pe["cB"], pe["qB"]

    common = {
        "xT": xT,
        "W1": W1.astype(np.float16),
        "W1T": np.ascontiguousarray(W1.T).astype(np.float16),
        "Amat": amat,
        "W2": W2.astype(np.float16),
        "W2T": np.ascontiguousarray(W2.T).astype(np.float16),
        "A2": a2,
        "asr": np.tile(asr_flat[None, :], (P, 1)).astype(np.float16),
        "b1r": np.tile(b1[None, :], (P, 1)).astype(np.float16),
        "b2r": np.tile(b2[None, :], (P, 1)).astype(np.float32),
    }
    in_maps = []
    for c in range(NCORES):
        m = dict(common)
        m["mask1"] = pe["mask1"][c]
        m["mask2"] = pe["mask2"][c]
        m["slo1"] = np.ascontiguousarray(pe["slo1"][c])
        shi1 = pe["shi1"][c]
        if shi1.shape[1] < 16:
            shi1 = np.zeros((P, 16), np.int16)
        m["shi1"] = np.ascontiguousarray(shi1)
        m["slo2"] = np.ascontiguousarray(pe["slo2"][c])
        shi2 = pe["shi2"][c]
        if shi2.shape[1] < 16:
            shi2 = np.zeros((P, 16), np.int16)
        m["shi2"] = np.ascontiguousarray(shi2)
        m["adlo"] = _wrap16(pe["adlo"][c])
        m["adhi"] = _wrap16(pe["adhi"][c])
        m["mlo"] = np.ascontiguousarray(pe["mlo"][c].reshape(NW, P).T)
        m["a2lo"] = _wrap16(pe["a2lo"][c])
        m["a2hi"] = _wrap16(pe["a2hi"][c])
        m["m2lo"] = np.ascontiguousarray(pe["m2lo"][c].reshape(NW, P).T)
        m["m2hi"] = np.ascontiguousarray(pe["m2hi"][c].reshape(NW, P).T)
        in_maps.append(m)
    return in_maps


TRACE = False
LAST_RESULTS = None
STOP = ""


def kernel(**inputs) -> np.ndarray:
    global LAST_RESULTS
    from concourse.bass_utils import run_bass_kernel_spmd

    cfg = {}
    if STOP:
        cfg["STOP"] = STOP
    in_maps = make_in_maps(inputs, cfg)
    nc = build_nc(cfg)
    if not nc.is_finalized():
        nc.finalize()
    res = run_bass_kernel_spmd(nc, in_maps, core_ids=list(range(NCORES)),
                               trace=TRACE)
    LAST_RESULTS = res
    outs = np.stack([res.results[c]["out"] for c in range(NCORES)])  # [8, NLPAD, 64]
    cB, qB = cfg["cB"], cfg["qB"]
    return outs[cB, qB].astype(np.float32)
